# revision 11
# baseline (speedup 1.0000x reference)
"""Trainium2 8-core tensor-parallel attention kernel (Bass/Tile).

Sharding: heads tensor-parallel across 8 cores (2 heads/core).
wq/wk/wv column-sharded by head, wo row-sharded; x replicated.
Chunked ReduceScatter (bf16) after the output projection; the host
concatenates the per-core row shards into the full output.

Fused single-phase design: Q/K/V stay SBUF-resident (no DRAM
roundtrip), attention for q-group g of batch b is emitted right after
the schunk that completes its K/V prefix, o-proj lags one unit so PE
never waits on the softmax-normalize chain, softmax row-sums run on
vector+gpsimd (not PE), V is transposed with DMA-transpose, and the
final ReduceScatter chunk is split 4x to shrink the drain tail.

Self-contained: hardcodes B=2, S=2048, DIM=2048, NH=16, HD=128.
"""
import math

import numpy as np

B, S_FULL, DIM, NH = 2, 2048, 2048, 16
HD = 128
N_CORES = 8
HPC = NH // N_CORES          # heads per core
OC = HPC * HD                # output channels per core (256)
DT = DIM // 128              # d-tiles (16)
SC_W = 512                   # schunk width (cols of flattened seq)
RS_ROWS = 512                # rows per ReduceScatter chunk

_CACHE = {}


def _build(S):
    """Build the 8-core SPMD Bass graph for sequence length S (B=2 fixed)."""
    import concourse.bass as bass
    import concourse.mybir as mybir
    import concourse.tile as tile
    from concourse import bacc

    from concourse import bass_isa

    fp32 = mybir.dt.float32
    bf16 = mybir.dt.bfloat16
    Exp = mybir.ActivationFunctionType.Exp
    Copy = mybir.ActivationFunctionType.Copy
    ADD = mybir.AluOpType.add
    RADD = bass_isa.ReduceOp.add

    FLAT = B * S                 # flattened rows (4096)
    NSC = FLAT // SC_W           # schunks (8)
    NQT = S // 128               # q/k tiles per batch (16)
    NQG = NQT // 4               # 512-col q-groups per batch (4)
    NCH = FLAT // RS_ROWS        # ReduceScatter chunks (8)
    SCALE = 1.0 / math.sqrt(HD)
    rg = [list(range(N_CORES))]

    nc = bacc.Bacc("TRN2", target_bir_lowering=False, debug=False,
                   num_devices=N_CORES)

    # ---- external parameters ----
    xt_d = nc.declare_dram_parameter("xt", [DIM, FLAT], bf16, isOutput=False)
    wqt_d = nc.declare_dram_parameter("wqt", [DIM, OC], bf16, isOutput=False)
    wkt_d = nc.declare_dram_parameter("wkt", [DIM, OC], bf16, isOutput=False)
    wvt_d = nc.declare_dram_parameter("wvt", [DIM, OC], bf16, isOutput=False)
    wot_d = nc.declare_dram_parameter("wot", [OC, DIM], bf16, isOutput=False)
    cos_d = nc.declare_dram_parameter("cos_t", [HD, S], bf16, isOutput=False)
    sin_d = nc.declare_dram_parameter("sin_t", [HD, S], bf16, isOutput=False)
    mdg_d = nc.declare_dram_parameter("mask_diag", [NQT, 128, 128], fp32, isOutput=False)
    rot_d = nc.declare_dram_parameter("rotp", [128, 128], bf16, isOutput=False)
    out_d = nc.declare_dram_parameter("out", [FLAT // N_CORES, DIM], bf16,
                                      isOutput=True)

    # ---- internal DRAM (o-proj partials + RS outputs) ----
    par_d = [nc.dram_tensor(f"partial_dram{c}", [RS_ROWS, DIM], bf16)
             for c in range(NCH)]
    rs_d = [nc.dram_tensor(f"rs_out{c}", [RS_ROWS // N_CORES, DIM], bf16)
            for c in range(NCH - 1)]
    rs7_d = [nc.dram_tensor(f"rs7_out{j}", [128 // N_CORES, DIM], bf16)
             for j in range(4)]

    from contextlib import ExitStack
    with tile.TileContext(nc) as tc:
        with ExitStack() as _stk:
            cpool = _stk.enter_context(tc.tile_pool(name="consts", bufs=1))
            qkvres = _stk.enter_context(tc.tile_pool(name="qkvres", bufs=1))
            xpool = _stk.enter_context(tc.tile_pool(name="xT", bufs=33))
            spool = _stk.enter_context(tc.tile_pool(name="cops", bufs=4))
            ptpool = _stk.enter_context(tc.tile_pool(name="probsT", bufs=3))
            accpool = _stk.enter_context(tc.tile_pool(name="accs", bufs=1))
            smpool = _stk.enter_context(tc.tile_pool(name="small", bufs=4))
            opool = _stk.enter_context(tc.tile_pool(name="outT", bufs=2))
            papool = _stk.enter_context(tc.tile_pool(name="partial", bufs=2))
            qkvps = _stk.enter_context(
                tc.tile_pool(name="qkvps", bufs=2, space="PSUM"))
            workps = _stk.enter_context(
                tc.tile_pool(name="workps", bufs=4, space="PSUM"))
            pops = _stk.enter_context(
                tc.tile_pool(name="pops", bufs=1, space="PSUM"))

            # ---- consts (gpsimd DMA queue; cheap triggers) ----
            wot_sb = cpool.tile([128, HPC, DIM], bf16)
            nc.gpsimd.dma_start(wot_sb[:], wot_d[:].rearrange("(h p) e -> p h e", p=128))
            cos_sb = cpool.tile([HD, S], bf16)
            nc.gpsimd.dma_start(cos_sb[:], cos_d[:])
            sin_sb = cpool.tile([HD, S], bf16)
            nc.gpsimd.dma_start(sin_sb[:], sin_d[:])
            mdg_sb = cpool.tile([128, NQT, 128], fp32)
            nc.gpsimd.dma_start(mdg_sb[:], mdg_d[:].rearrange("t p k -> p t k"))
            rot_sb = cpool.tile([128, 128], bf16)
            nc.gpsimd.dma_start(rot_sb[:], rot_d[:])

            # ---- weights + first x chunk, interleaved in consumption order
            w_sb = {}
            for nm in ("q", "k", "v"):
                w_sb[nm] = qkvres.tile([128, DT, OC], bf16, tag=f"w{nm}", name=f"w{nm}")

            xts = {}  # (sc, dt) -> tile

            def load_x(sc):
                for dt in range(DT):
                    xt = xpool.tile([128, SC_W], bf16, tag="xt", name=f"xt{sc}_{dt}")
                    eng = nc.sync if dt % 2 == 0 else nc.gpsimd
                    eng.dma_start(
                        xt[:], xt_d[dt * 128:(dt + 1) * 128,
                                    sc * SC_W:(sc + 1) * SC_W])
                    xts[(sc, dt)] = xt

            for dt in range(DT):
                nc.sync.dma_start(w_sb["q"][:, dt, :],
                                  wqt_d[dt * 128:(dt + 1) * 128, :])
                xt = xpool.tile([128, SC_W], bf16, tag="xt", name=f"xt0_{dt}")
                nc.sync.dma_start(xt[:], xt_d[dt * 128:(dt + 1) * 128, 0:SC_W])
                xts[(0, dt)] = xt
            for dt in range(DT):
                nc.sync.dma_start(w_sb["k"][:, dt, :],
                                  wkt_d[dt * 128:(dt + 1) * 128, :])
            for dt in range(DT):
                nc.sync.dma_start(w_sb["v"][:, dt, :],
                                  wvt_d[dt * 128:(dt + 1) * 128, :])

            # ---- SBUF-resident q/k/v per (batch, head) ----
            qT = {(b, h): qkvres.tile([128, S], bf16, tag=f"qT{b}{h}", name=f"qT{b}{h}")
                  for b in range(B) for h in range(HPC)}
            kT = {(b, h): qkvres.tile([128, S], bf16, tag=f"kT{b}{h}", name=f"kT{b}{h}")
                  for b in range(B) for h in range(HPC)}
            vN = {(b, h): qkvres.tile([128, NQT, HD], bf16, tag=f"vN{b}{h}",
                                      name=f"vN{b}{h}")
                  for b in range(B) for h in range(HPC)}

            # =========== emission helpers ===========

            def emit_qkv_chunk(sc):
                """QKV projections + RoPE for one 512-col schunk."""
                bb, c0 = divmod(sc * SC_W, S)
                s0 = c0  # position offset within batch
                # order: q(h0) q(h1) k(h0) k(h1) v(h0) v(h1); rope matmul of
                # each chain is emitted one chain later so its scalar-copy
                # input is ready without stalling PE.
                chains = [(t, h) for t in ("q", "k", "v") for h in range(HPC)]
                pend = []  # rope matmuls pending emission: (t, h, ps)
                ps_of = {}
                for ci, (t, h) in enumerate(chains):
                    ps = qkvps.tile([128, SC_W], fp32, tag="qkv", name=f"ps_{t}{h}")
                    ps_of[(t, h)] = ps
                    for dt in range(DT):
                        nc.tensor.matmul(
                            ps[:],
                            w_sb[t][:, dt, h * HD:(h + 1) * HD],
                            xts[(sc, dt)][:],
                            start=(dt == 0), stop=(dt == DT - 1))
                    if t in ("q", "k"):
                        # PSUM -> SBUF bf16 (+1/sqrt(hd) for q)
                        til = spool.tile([128, SC_W], bf16, tag="til", name=f"til{t}{h}")
                        nc.scalar.activation(til[:], ps[:], Copy,
                                             scale=SCALE if t == "q" else 1.0)
                        pend.append((t, h, til))
                    else:
                        vb = spool.tile([128, SC_W], bf16, tag="vb", name=f"vb{h}")
                        nc.scalar.copy(vb[:], ps[:])
                        for vt in range(SC_W // 128):
                            ktile = c0 // 128 + vt
                            nc.sync.dma_start_transpose(
                                vN[(bb, h)][:, ktile, :],
                                vb[:, vt * 128:(vt + 1) * 128])
                    # emit the rope matmul of the chain before last, so its
                    # til copy has a full chain's worth of slack
                    if ci >= 1 and pend:
                        _emit_rope(bb, s0, *pend.pop(0))
                for args in pend:
                    _emit_rope(bb, s0, *args)

            def _emit_rope(bb, s0, t, h, til):
                rp = workps.tile([128, SC_W], fp32, tag="work", name=f"rot{t}{h}")
                nc.tensor.matmul(rp[:], rot_sb[:], til[:], start=True, stop=True)
                dst = qT[(bb, h)] if t == "q" else kT[(bb, h)]
                t1 = spool.tile([128, SC_W], bf16, tag="t1", name=f"t1{t}{h}")
                nc.vector.tensor_mul(t1[:], til[:], cos_sb[:, s0:s0 + SC_W])
                hat = spool.tile([128, SC_W], bf16, tag="hat", name=f"hat{t}{h}")
                nc.vector.tensor_mul(hat[:], rp[:], sin_sb[:, s0:s0 + SC_W])
                nc.vector.tensor_add(dst[:, s0:s0 + SC_W], hat[:], t1[:])

            oT_of = {}  # unit -> {h: oT tile}

            def emit_attn_unit(bb, qg):
                """Attention for 512 q-cols (group qg) of batch bb."""
                kmax = qg * 4 + 3
                po = {h: pops.tile([128, 512], fp32, tag=f"po{h}", name=f"po{h}")
                      for h in range(HPC)}
                acc_v = {h: accpool.tile([128, 512], fp32, tag=f"av{h}",
                                         name=f"accv{h}") for h in range(HPC)}
                acc_g = {h: accpool.tile([128, 512], fp32, tag=f"ag{h}",
                                         name=f"accg{h}") for h in range(HPC)}
                pt_hist = {h: {} for h in range(HPC)}

                def rowsum(h, kt, qlo, n):
                    # qg==0 has shrinking windows from kt=1 on; keep those
                    # units entirely on the vector accumulator.
                    pt = pt_hist[h][kt]
                    if qg == 0:
                        if kt == 0:
                            nc.vector.tensor_copy(acc_v[h][:], pt[:, :n])
                        else:
                            nc.vector.tensor_add(acc_v[h][:, qlo:512],
                                                 acc_v[h][:, qlo:512], pt[:, :n])
                        return
                    if kt == 0:
                        nc.vector.tensor_copy(acc_v[h][:], pt[:, :n])
                    elif kt == 1:
                        nc.gpsimd.tensor_copy(acc_g[h][:], pt[:, :n])
                    elif kt % 2 == 0:
                        nc.vector.tensor_add(acc_v[h][:, qlo:512],
                                             acc_v[h][:, qlo:512], pt[:, :n])
                    else:
                        nc.gpsimd.tensor_add(acc_g[h][:, qlo:512],
                                             acc_g[h][:, qlo:512], pt[:, :n])

                for kt in range(kmax + 1):
                    qlo = max(0, kt - qg * 4) * 128
                    n = 512 - qlo
                    for h in range(HPC):
                        sp = workps.tile([128, 512], fp32, tag="work", name="sp")
                        nc.tensor.matmul(
                            sp[:, :n],
                            kT[(bb, h)][:, kt * 128:(kt + 1) * 128],
                            qT[(bb, h)][:, qg * 512 + qlo:(qg + 1) * 512],
                            start=True, stop=True)
                        if kt >= qg * 4:  # diagonal block: causal mask
                            nc.vector.tensor_add(
                                sp[:, 0:128], sp[:, 0:128], mdg_sb[:, kt, :])
                        pt = ptpool.tile([128, 512], bf16, tag=f"pT{h}",
                                         name=f"pT{h}")
                        pt_hist[h][kt] = pt
                        nc.scalar.activation(pt[:, :n], sp[:, :n], Exp)
                        rowsum(h, kt, qlo, n)
                    if kt >= 1:
                        kl = kt - 1
                        ql2 = max(0, kl - qg * 4) * 128
                        n2 = 512 - ql2
                        for h in range(HPC):
                            nc.tensor.matmul(
                                po[h][:, ql2:512], vN[(bb, h)][:, kl, :],
                                pt_hist[h][kl][:, :n2],
                                start=(kl == 0), stop=False)
                for h in range(HPC):
                    nc.tensor.matmul(
                        po[h][:, 384:512], vN[(bb, h)][:, kmax, :],
                        pt_hist[h][kmax][:, :128], start=False, stop=True)

                # softmax denominators off the critical PE path
                oT_of[(bb, qg)] = {}
                for h in range(HPC):
                    if qg > 0:
                        nc.vector.tensor_add(acc_v[h][:], acc_v[h][:],
                                             acc_g[h][:])
                    sums = smpool.tile([128, 512], fp32, tag="sums",
                                       name="sums", bufs=2)
                    nc.gpsimd.partition_all_reduce(sums[:], acc_v[h][:],
                                                   channels=128,
                                                   reduce_op=RADD)
                    rbc = smpool.tile([128, 512], fp32, tag="rbc", name="rbc",
                                      bufs=2)
                    nc.vector.reciprocal_approx_fast(rbc[:], sums[:])
                    ot = opool.tile([128, 512], bf16, tag=f"oT{h}", name=f"oT{h}")
                    nc.vector.tensor_mul(ot[:], po[h][:], rbc[:])
                    oT_of[(bb, qg)][h] = ot

            def emit_oproj(bb, qg):
                """O-projection + ReduceScatter for one 512-row unit."""
                ot = oT_of.pop((bb, qg))
                chx = bb * NQG + qg
                last = chx == NCH - 1
                for st in range(4):
                    par = papool.tile([128, DIM], bf16, tag="par", name="par")
                    for ec in range(4):
                        pp = workps.tile([128, 512], fp32, tag="work", name="pp")
                        for h in range(HPC):
                            nc.tensor.matmul(
                                pp[:],
                                ot[h][:, st * 128:(st + 1) * 128],
                                wot_sb[:, h, ec * 512:(ec + 1) * 512],
                                start=(h == 0), stop=(h == HPC - 1))
                        if ec % 2 == 0:
                            nc.scalar.copy(par[:, ec * 512:(ec + 1) * 512], pp[:])
                        else:
                            nc.vector.tensor_copy(par[:, ec * 512:(ec + 1) * 512], pp[:])
                    nc.sync.dma_start(par_d[chx][st * 128:(st + 1) * 128, :], par[:])
                    if last:
                        # final unit: 128-row collectives to shrink the tail
                        nc.gpsimd.collective_compute(
                            "ReduceScatter", ADD, replica_groups=rg,
                            ins=[par_d[chx][st * 128:(st + 1) * 128, :]],
                            outs=[rs7_d[st][:]])
                        nc.gpsimd.dma_start(
                            out_d[(NCH - 1) * 64 + st * 16:
                                  (NCH - 1) * 64 + (st + 1) * 16, :],
                            rs7_d[st][:])
                if not last:
                    nc.gpsimd.collective_compute(
                        "ReduceScatter", ADD, replica_groups=rg,
                        ins=[par_d[chx][:]],
                        outs=[rs_d[chx][:]])
                    nc.gpsimd.dma_start(out_d[chx * 64:(chx + 1) * 64, :],
                                        rs_d[chx][:])

            # =========== main schedule ===========
            prev_unit = None
            for sc in range(NSC):
                if sc + 1 < NSC:
                    load_x(sc + 1)
                emit_qkv_chunk(sc)
                if prev_unit is not None:
                    emit_oproj(*prev_unit)
                bb, qg = sc // NQG, sc % NQG
                emit_attn_unit(bb, qg)
                prev_unit = (bb, qg)
            emit_oproj(*prev_unit)

    nc.compile()
    return nc


def _get_nc(S):
    if S not in _CACHE:
        _CACHE[S] = _build(S)
    return _CACHE[S]


def make_inputs(x, freqs_cis, mask, wq, wk, wv, wo):
    """Host-side sharding / layout prep. Returns in_maps for 8 cores."""
    S = x.shape[1]
    flat_xt = np.ascontiguousarray(np.asarray(x, np.float32).reshape(B * S, DIM).T)
    cos = np.asarray(freqs_cis[..., 0], np.float32)   # [S, HD/2]
    sin = np.asarray(freqs_cis[..., 1], np.float32)
    cos_t = np.ascontiguousarray(np.repeat(cos.T, 2, axis=0))  # [HD, S]
    sin_t = np.ascontiguousarray(np.repeat(sin.T, 2, axis=0))
    m = np.asarray(mask, np.float32)[0, 0]
    nqt = S // 128
    mask_diag = np.ascontiguousarray(
        np.stack([m[i * 128:(i + 1) * 128, i * 128:(i + 1) * 128].T
                  for i in range(nqt)]))
    import ml_dtypes
    bf = ml_dtypes.bfloat16
    flat_xt = flat_xt.astype(bf)
    cos_t = cos_t.astype(bf)
    sin_t = sin_t.astype(bf)
    P = np.zeros((128, 128), np.float32)
    for j in range(64):
        P[2 * j, 2 * j + 1] = -1.0
        P[2 * j + 1, 2 * j] = 1.0
    rotp = np.ascontiguousarray(P.T)

    in_maps = []
    for c in range(N_CORES):
        r = slice(c * OC, (c + 1) * OC)
        in_maps.append({
            "xt": flat_xt,
            "wqt": np.ascontiguousarray(np.asarray(wq, np.float32)[r, :].T).astype(bf),
            "wkt": np.ascontiguousarray(np.asarray(wk, np.float32)[r, :].T).astype(bf),
            "wvt": np.ascontiguousarray(np.asarray(wv, np.float32)[r, :].T).astype(bf),
            "wot": np.ascontiguousarray(np.asarray(wo, np.float32)[:, r].T).astype(bf),
            "cos_t": cos_t,
            "sin_t": sin_t,
            "mask_diag": mask_diag,
            "rotp": rotp.astype(bf),
        })
    return in_maps


def assemble(results, S):
    """Undo the per-core ReduceScatter sharding into the full output.

    Chunks 0..6 are 512 rows (64 rows/core); the final 512 rows were
    reduced as four 128-row chunks (16 rows/core each).
    """
    nch = B * S // RS_ROWS
    full = np.empty((B * S, DIM), np.float32)
    for c in range(N_CORES):
        o = np.asarray(results[c]["out"], np.float32)  # [512, DIM]
        for chx in range(nch - 1):
            full[chx * 512 + c * 64:chx * 512 + (c + 1) * 64] = \
                o[chx * 64:(chx + 1) * 64]
        for j in range(4):
            r0 = (nch - 1) * 512 + j * 128 + c * 16
            full[r0:r0 + 16] = o[(nch - 1) * 64 + j * 16:
                                 (nch - 1) * 64 + (j + 1) * 16]
    return full.reshape(B, S, DIM)


def kernel(x, start_pos, freqs_cis, mask, wq, wk, wv, wo):
    from concourse.bass_utils import run_bass_kernel_spmd
    S = x.shape[1]
    nc = _get_nc(S)
    in_maps = make_inputs(x, freqs_cis, mask, wq, wk, wv, wo)
    res = run_bass_kernel_spmd(nc, in_maps, core_ids=list(range(N_CORES)))
    return assemble(res.results, S)


# revision 21
# speedup vs baseline: 1.0953x; 1.0953x over previous
"""Trainium2 8-core tensor-parallel attention kernel (Bass/Tile).

Sharding: heads tensor-parallel across 8 cores (2 heads/core).
wq/wk/wv column-sharded by head, wo row-sharded; x replicated.
Chunked ReduceScatter (bf16) after the output projection; the host
concatenates the per-core row shards into the full output.

Fused single-phase design: Q/K/V stay SBUF-resident (no DRAM
roundtrip), attention for q-group g of batch b is emitted right after
the schunk that completes its K/V prefix, o-proj lags one unit so PE
never waits on the softmax-normalize chain, softmax row-sums run on
vector+gpsimd (not PE), V is transposed with DMA-transpose, and the
final ReduceScatter chunk is split 4x to shrink the drain tail.

Self-contained: hardcodes B=2, S=2048, DIM=2048, NH=16, HD=128.
"""
import math

import numpy as np

B, S_FULL, DIM, NH = 2, 2048, 2048, 16
HD = 128
N_CORES = 8
HPC = NH // N_CORES          # heads per core
OC = HPC * HD                # output channels per core (256)
DT = DIM // 128              # d-tiles (16)
SC_W = 512                   # schunk width (cols of flattened seq)
RS_ROWS = 512                # rows per ReduceScatter chunk

_CACHE = {}


def _build(S):
    """Build the 8-core SPMD Bass graph for sequence length S (B=2 fixed)."""
    import concourse.bass as bass
    import concourse.mybir as mybir
    import concourse.tile as tile
    from concourse import bacc

    from concourse import bass_isa

    fp32 = mybir.dt.float32
    bf16 = mybir.dt.bfloat16
    Exp = mybir.ActivationFunctionType.Exp
    Copy = mybir.ActivationFunctionType.Copy
    ADD = mybir.AluOpType.add
    RADD = bass_isa.ReduceOp.add

    FLAT = B * S                 # flattened rows (4096)
    NSC = FLAT // SC_W           # schunks (8)
    NQT = S // 128               # q/k tiles per batch (16)
    NQG = NQT // 4               # 512-col q-groups per batch (4)
    NCH = FLAT // RS_ROWS        # ReduceScatter chunks (8)
    SCALE = 1.0 / math.sqrt(HD)
    rg = [list(range(N_CORES))]

    nc = bacc.Bacc("TRN2", target_bir_lowering=False, debug=False,
                   num_devices=N_CORES)

    # ---- external parameters ----
    xt_d = nc.declare_dram_parameter("xt", [DIM, FLAT], bf16, isOutput=False)
    wqt_d = nc.declare_dram_parameter("wqt", [DIM, OC], bf16, isOutput=False)
    wkt_d = nc.declare_dram_parameter("wkt", [DIM, OC], bf16, isOutput=False)
    wvt_d = nc.declare_dram_parameter("wvt", [DIM, OC], bf16, isOutput=False)
    wot_d = nc.declare_dram_parameter("wot", [OC, DIM], bf16, isOutput=False)
    cos_d = nc.declare_dram_parameter("cos_t", [HD, S], bf16, isOutput=False)
    sin_d = nc.declare_dram_parameter("sin_t", [HD, S], bf16, isOutput=False)
    mdg_d = nc.declare_dram_parameter("mask_diag", [NQT, 128, 128], fp32, isOutput=False)
    rot_d = nc.declare_dram_parameter("rotp", [128, 128], bf16, isOutput=False)
    out_d = nc.declare_dram_parameter("out", [FLAT // N_CORES, DIM], bf16,
                                      isOutput=True)

    # ---- internal DRAM (o-proj partials + RS outputs) ----
    par_d = [nc.dram_tensor(f"partial_dram{c}", [RS_ROWS, DIM], bf16)
             for c in range(NCH)]
    rs_d = [nc.dram_tensor(f"rs_out{c}", [RS_ROWS // N_CORES, DIM], bf16)
            for c in range(NCH)]

    from contextlib import ExitStack
    with tile.TileContext(nc) as tc:
        with ExitStack() as _stk:
            cpool = _stk.enter_context(tc.tile_pool(name="consts", bufs=1))
            qkvres = _stk.enter_context(tc.tile_pool(name="qkvres", bufs=1))
            xpool = _stk.enter_context(tc.tile_pool(name="xT", bufs=33))
            spool = _stk.enter_context(tc.tile_pool(name="cops", bufs=4))
            ptpool = _stk.enter_context(tc.tile_pool(name="probsT", bufs=3))
            accpool = _stk.enter_context(tc.tile_pool(name="accs", bufs=1))
            smpool = _stk.enter_context(tc.tile_pool(name="small", bufs=4))
            opool = _stk.enter_context(tc.tile_pool(name="outT", bufs=2))
            papool = _stk.enter_context(tc.tile_pool(name="partial", bufs=2))
            qkvps = _stk.enter_context(
                tc.tile_pool(name="qkvps", bufs=2, space="PSUM"))
            workps = _stk.enter_context(
                tc.tile_pool(name="workps", bufs=4, space="PSUM"))
            pops = _stk.enter_context(
                tc.tile_pool(name="pops", bufs=1, space="PSUM"))

            # ---- consts (gpsimd DMA queue; cheap triggers) ----
            wot_sb = cpool.tile([128, HPC, DIM], bf16)
            nc.gpsimd.dma_start(wot_sb[:], wot_d[:].rearrange("(h p) e -> p h e", p=128))
            cos_sb = cpool.tile([HD, S], bf16)
            nc.gpsimd.dma_start(cos_sb[:], cos_d[:])
            sin_sb = cpool.tile([HD, S], bf16)
            nc.gpsimd.dma_start(sin_sb[:], sin_d[:])
            mdg_sb = cpool.tile([128, NQT, 128], fp32)
            nc.gpsimd.dma_start(mdg_sb[:], mdg_d[:].rearrange("t p k -> p t k"))
            rot_sb = cpool.tile([128, 128], bf16)
            nc.gpsimd.dma_start(rot_sb[:], rot_d[:])

            # ---- weights + first x chunk, interleaved in consumption order
            w_sb = {}
            for nm in ("q", "k", "v"):
                w_sb[nm] = qkvres.tile([128, DT, OC], bf16, tag=f"w{nm}", name=f"w{nm}")

            xts = {}  # (sc, dt) -> tile

            def load_x(sc):
                for dt in range(DT):
                    xt = xpool.tile([128, SC_W], bf16, tag="xt", name=f"xt{sc}_{dt}")
                    nc.sync.dma_start(
                        xt[:], xt_d[dt * 128:(dt + 1) * 128,
                                    sc * SC_W:(sc + 1) * SC_W])
                    xts[(sc, dt)] = xt

            for dt in range(DT):
                nc.sync.dma_start(w_sb["q"][:, dt, :],
                                  wqt_d[dt * 128:(dt + 1) * 128, :])
                xt = xpool.tile([128, SC_W], bf16, tag="xt", name=f"xt0_{dt}")
                nc.sync.dma_start(xt[:], xt_d[dt * 128:(dt + 1) * 128, 0:SC_W])
                xts[(0, dt)] = xt
            for dt in range(DT):
                nc.sync.dma_start(w_sb["k"][:, dt, :],
                                  wkt_d[dt * 128:(dt + 1) * 128, :])
            for dt in range(DT):
                nc.sync.dma_start(w_sb["v"][:, dt, :],
                                  wvt_d[dt * 128:(dt + 1) * 128, :])

            # ---- SBUF-resident q/k/v per (batch, head) ----
            qT = {(b, h): qkvres.tile([128, S], bf16, tag=f"qT{b}{h}", name=f"qT{b}{h}")
                  for b in range(B) for h in range(HPC)}
            kT = {(b, h): qkvres.tile([128, S], bf16, tag=f"kT{b}{h}", name=f"kT{b}{h}")
                  for b in range(B) for h in range(HPC)}
            vN = {(b, h): qkvres.tile([128, NQT, HD], bf16, tag=f"vN{b}{h}",
                                      name=f"vN{b}{h}")
                  for b in range(B) for h in range(HPC)}

            # =========== emission helpers ===========

            def emit_qkv_chunk(sc):
                """QKV projections + RoPE for one 512-col schunk."""
                bb, c0 = divmod(sc * SC_W, S)
                s0 = c0  # position offset within batch
                # order: q(h0) q(h1) k(h0) k(h1) v(h0) v(h1); rope matmul of
                # each chain is emitted one chain later so its scalar-copy
                # input is ready without stalling PE.
                chains = [(t, h) for t in ("q", "k", "v") for h in range(HPC)]
                pend = []  # rope matmuls pending emission: (t, h, ps)
                ps_of = {}
                for ci, (t, h) in enumerate(chains):
                    ps = qkvps.tile([128, SC_W], fp32, tag="qkv", name=f"ps_{t}{h}")
                    ps_of[(t, h)] = ps
                    for dt in range(DT):
                        nc.tensor.matmul(
                            ps[:],
                            w_sb[t][:, dt, h * HD:(h + 1) * HD],
                            xts[(sc, dt)][:],
                            start=(dt == 0), stop=(dt == DT - 1))
                    if t in ("q", "k"):
                        # PSUM -> SBUF bf16 (+1/sqrt(hd) for q)
                        til = spool.tile([128, SC_W], bf16, tag="til", name=f"til{t}{h}")
                        nc.scalar.activation(til[:], ps[:], Copy,
                                             scale=SCALE if t == "q" else 1.0)
                        pend.append((t, h, til))
                    else:
                        vb = spool.tile([128, SC_W], bf16, tag="vb", name=f"vb{h}")
                        nc.scalar.copy(vb[:], ps[:])
                        for vt in range(SC_W // 128):
                            ktile = c0 // 128 + vt
                            nc.sync.dma_start_transpose(
                                vN[(bb, h)][:, ktile, :],
                                vb[:, vt * 128:(vt + 1) * 128])
                    # emit the rope matmul of the chain before last, so its
                    # til copy has a full chain's worth of slack
                    if ci >= 1 and pend:
                        _emit_rope(bb, s0, *pend.pop(0))
                for args in pend:
                    _emit_rope(bb, s0, *args)

            def _emit_rope(bb, s0, t, h, til):
                rp = workps.tile([128, SC_W], fp32, tag="work", name=f"rot{t}{h}")
                nc.tensor.matmul(rp[:], rot_sb[:], til[:], start=True, stop=True)
                dst = qT[(bb, h)] if t == "q" else kT[(bb, h)]
                t1 = spool.tile([128, SC_W], bf16, tag="t1", name=f"t1{t}{h}")
                nc.vector.tensor_mul(t1[:], til[:], cos_sb[:, s0:s0 + SC_W])
                hat = spool.tile([128, SC_W], bf16, tag="hat", name=f"hat{t}{h}")
                nc.vector.tensor_mul(hat[:], rp[:], sin_sb[:, s0:s0 + SC_W])
                nc.vector.tensor_add(dst[:, s0:s0 + SC_W], hat[:], t1[:])

            oT_of = {}  # unit -> {h: oT tile}

            def emit_attn_unit(bb, qg):
                """Attention for 512 q-cols (group qg) of batch bb."""
                kmax = qg * 4 + 3
                po = {h: pops.tile([128, 512], fp32, tag=f"po{h}", name=f"po{h}")
                      for h in range(HPC)}
                acc_v = {h: accpool.tile([128, 512], bf16, tag=f"av{h}",
                                         name=f"accv{h}") for h in range(HPC)}
                acc_g = {h: accpool.tile([128, 512], bf16, tag=f"ag{h}",
                                         name=f"accg{h}") for h in range(HPC)}
                pt_hist = {h: {} for h in range(HPC)}

                def rowsum(h, kt, qlo, n):
                    # qg==0 has shrinking windows from kt=1 on; keep those
                    # units entirely on the vector accumulator.
                    pt = pt_hist[h][kt]
                    if qg == 0:
                        if kt == 0:
                            nc.vector.tensor_copy(acc_v[h][:], pt[:, :n])
                        else:
                            nc.vector.tensor_add(acc_v[h][:, qlo:512],
                                                 acc_v[h][:, qlo:512], pt[:, :n])
                        return
                    if kt == 0:
                        nc.vector.tensor_copy(acc_v[h][:], pt[:, :n])
                    elif kt == 1:
                        nc.gpsimd.tensor_copy(acc_g[h][:], pt[:, :n])
                    elif kt % 2 == 0:
                        nc.vector.tensor_add(acc_v[h][:, qlo:512],
                                             acc_v[h][:, qlo:512], pt[:, :n])
                    else:
                        nc.gpsimd.tensor_add(acc_g[h][:, qlo:512],
                                             acc_g[h][:, qlo:512], pt[:, :n])

                for kt in range(kmax + 1):
                    qlo = max(0, kt - qg * 4) * 128
                    n = 512 - qlo
                    for h in range(HPC):
                        sp = workps.tile([128, 512], fp32, tag="work", name="sp")
                        nc.tensor.matmul(
                            sp[:, :n],
                            kT[(bb, h)][:, kt * 128:(kt + 1) * 128],
                            qT[(bb, h)][:, qg * 512 + qlo:(qg + 1) * 512],
                            start=True, stop=True)
                        if kt >= qg * 4:  # diagonal block: causal mask
                            nc.vector.tensor_add(
                                sp[:, 0:128], sp[:, 0:128], mdg_sb[:, kt, :])
                        pt = ptpool.tile([128, 512], bf16, tag=f"pT{h}",
                                         name=f"pT{h}")
                        pt_hist[h][kt] = pt
                        nc.scalar.activation(pt[:, :n], sp[:, :n], Exp)
                        rowsum(h, kt, qlo, n)
                    if kt >= 1:
                        kl = kt - 1
                        ql2 = max(0, kl - qg * 4) * 128
                        n2 = 512 - ql2
                        for h in range(HPC):
                            nc.tensor.matmul(
                                po[h][:, ql2:512], vN[(bb, h)][:, kl, :],
                                pt_hist[h][kl][:, :n2],
                                start=(kl == 0), stop=False)
                for h in range(HPC):
                    nc.tensor.matmul(
                        po[h][:, 384:512], vN[(bb, h)][:, kmax, :],
                        pt_hist[h][kmax][:, :128], start=False, stop=True)

                # softmax denominators off the critical PE path
                oT_of[(bb, qg)] = {}
                for h in range(HPC):
                    if qg > 0:
                        nc.vector.tensor_add(acc_v[h][:], acc_v[h][:],
                                             acc_g[h][:])
                    sums = smpool.tile([128, 512], fp32, tag="sums",
                                       name="sums", bufs=2)
                    nc.gpsimd.partition_all_reduce(sums[:], acc_v[h][:],
                                                   channels=128,
                                                   reduce_op=RADD)
                    rbc = smpool.tile([128, 512], fp32, tag="rbc", name="rbc",
                                      bufs=2)
                    nc.vector.reciprocal_approx_fast(rbc[:], sums[:])
                    ot = opool.tile([128, 512], bf16, tag=f"oT{h}", name=f"oT{h}")
                    nc.vector.tensor_mul(ot[:], po[h][:], rbc[:])
                    oT_of[(bb, qg)][h] = ot

            def emit_oproj(bb, qg):
                """O-projection + ReduceScatter for one 512-row unit."""
                ot = oT_of.pop((bb, qg))
                chx = bb * NQG + qg
                for st in range(4):
                    par = papool.tile([128, DIM], bf16, tag="par", name="par")
                    for ec in range(4):
                        pp = workps.tile([128, 512], fp32, tag="work", name="pp")
                        for h in range(HPC):
                            nc.tensor.matmul(
                                pp[:],
                                ot[h][:, st * 128:(st + 1) * 128],
                                wot_sb[:, h, ec * 512:(ec + 1) * 512],
                                start=(h == 0), stop=(h == HPC - 1))
                        if ec % 2 == 0:
                            nc.scalar.copy(par[:, ec * 512:(ec + 1) * 512], pp[:])
                        else:
                            nc.vector.tensor_copy(par[:, ec * 512:(ec + 1) * 512], pp[:])
                    nc.sync.dma_start(par_d[chx][st * 128:(st + 1) * 128, :], par[:])
                nc.gpsimd.collective_compute(
                    "ReduceScatter", ADD, replica_groups=rg,
                    ins=[par_d[chx][:]],
                    outs=[rs_d[chx][:]])

            # =========== main schedule ===========
            prev_unit = None
            for sc in range(NSC):
                if sc + 1 < NSC:
                    load_x(sc + 1)
                emit_qkv_chunk(sc)
                if prev_unit is not None:
                    emit_oproj(*prev_unit)
                bb, qg = sc // NQG, sc % NQG
                emit_attn_unit(bb, qg)
                prev_unit = (bb, qg)
            emit_oproj(*prev_unit)
            # rs -> out copies all at the end: chunks 0..6 are long done
            # (no queue blocking); only chunk 7's copy rides the RS tail.
            for chx in range(NCH):
                nc.gpsimd.dma_start(out_d[chx * 64:(chx + 1) * 64, :],
                                    rs_d[chx][:])

    nc.compile()
    return nc


def _get_nc(S):
    if S not in _CACHE:
        _CACHE[S] = _build(S)
    return _CACHE[S]


def make_inputs(x, freqs_cis, mask, wq, wk, wv, wo):
    """Host-side sharding / layout prep. Returns in_maps for 8 cores."""
    S = x.shape[1]
    flat_xt = np.ascontiguousarray(np.asarray(x, np.float32).reshape(B * S, DIM).T)
    cos = np.asarray(freqs_cis[..., 0], np.float32)   # [S, HD/2]
    sin = np.asarray(freqs_cis[..., 1], np.float32)
    cos_t = np.ascontiguousarray(np.repeat(cos.T, 2, axis=0))  # [HD, S]
    sin_t = np.ascontiguousarray(np.repeat(sin.T, 2, axis=0))
    m = np.asarray(mask, np.float32)[0, 0]
    nqt = S // 128
    mask_diag = np.ascontiguousarray(
        np.stack([m[i * 128:(i + 1) * 128, i * 128:(i + 1) * 128].T
                  for i in range(nqt)]))
    import ml_dtypes
    bf = ml_dtypes.bfloat16
    flat_xt = flat_xt.astype(bf)
    cos_t = cos_t.astype(bf)
    sin_t = sin_t.astype(bf)
    P = np.zeros((128, 128), np.float32)
    for j in range(64):
        P[2 * j, 2 * j + 1] = -1.0
        P[2 * j + 1, 2 * j] = 1.0
    rotp = np.ascontiguousarray(P.T)

    in_maps = []
    for c in range(N_CORES):
        r = slice(c * OC, (c + 1) * OC)
        in_maps.append({
            "xt": flat_xt,
            "wqt": np.ascontiguousarray(np.asarray(wq, np.float32)[r, :].T).astype(bf),
            "wkt": np.ascontiguousarray(np.asarray(wk, np.float32)[r, :].T).astype(bf),
            "wvt": np.ascontiguousarray(np.asarray(wv, np.float32)[r, :].T).astype(bf),
            "wot": np.ascontiguousarray(np.asarray(wo, np.float32)[:, r].T).astype(bf),
            "cos_t": cos_t,
            "sin_t": sin_t,
            "mask_diag": mask_diag,
            "rotp": rotp.astype(bf),
        })
    return in_maps


def assemble(results, S):
    """Undo the per-core ReduceScatter sharding into the full output."""
    nch = B * S // RS_ROWS
    full = np.empty((B * S, DIM), np.float32)
    for c in range(N_CORES):
        o = np.asarray(results[c]["out"], np.float32)  # [512, DIM]
        for chx in range(nch):
            full[chx * 512 + c * 64:chx * 512 + (c + 1) * 64] = \
                o[chx * 64:(chx + 1) * 64]
    return full.reshape(B, S, DIM)


def kernel(x, start_pos, freqs_cis, mask, wq, wk, wv, wo):
    from concourse.bass_utils import run_bass_kernel_spmd
    S = x.shape[1]
    nc = _get_nc(S)
    in_maps = make_inputs(x, freqs_cis, mask, wq, wk, wv, wo)
    res = run_bass_kernel_spmd(nc, in_maps, core_ids=list(range(N_CORES)))
    return assemble(res.results, S)


# revision 23
# speedup vs baseline: 1.1207x; 1.0231x over previous
"""Trainium2 8-core tensor-parallel attention kernel (Bass/Tile).

Sharding: heads tensor-parallel across 8 cores (2 heads/core).
wq/wk/wv column-sharded by head, wo row-sharded; x replicated.
Chunked ReduceScatter (bf16) after the output projection; the host
concatenates the per-core row shards into the full output.

Fused single-phase design: Q/K/V stay SBUF-resident (no DRAM
roundtrip), attention for q-group g of batch b is emitted right after
the schunk that completes its K/V prefix, o-proj lags one unit so PE
never waits on the softmax-normalize chain, softmax row-sums run on
vector+gpsimd (not PE), V is transposed with DMA-transpose, and the
final ReduceScatter chunk is split 4x to shrink the drain tail.

Self-contained: hardcodes B=2, S=2048, DIM=2048, NH=16, HD=128.
"""
import math

import numpy as np

B, S_FULL, DIM, NH = 2, 2048, 2048, 16
HD = 128
N_CORES = 8
HPC = NH // N_CORES          # heads per core
OC = HPC * HD                # output channels per core (256)
DT = DIM // 128              # d-tiles (16)
SC_W = 512                   # schunk width (cols of flattened seq)
RS_ROWS = 512                # rows per ReduceScatter chunk

_CACHE = {}


def _build(S):
    """Build the 8-core SPMD Bass graph for sequence length S (B=2 fixed)."""
    import concourse.bass as bass
    import concourse.mybir as mybir
    import concourse.tile as tile
    from concourse import bacc

    from concourse import bass_isa

    fp32 = mybir.dt.float32
    bf16 = mybir.dt.bfloat16
    Exp = mybir.ActivationFunctionType.Exp
    Copy = mybir.ActivationFunctionType.Copy
    ADD = mybir.AluOpType.add
    RADD = bass_isa.ReduceOp.add

    FLAT = B * S                 # flattened rows (4096)
    NSC = FLAT // SC_W           # schunks (8)
    NQT = S // 128               # q/k tiles per batch (16)
    NQG = NQT // 4               # 512-col q-groups per batch (4)
    NCH = FLAT // RS_ROWS        # ReduceScatter chunks (8)
    SCALE = 1.0 / math.sqrt(HD)
    rg = [list(range(N_CORES))]

    nc = bacc.Bacc("TRN2", target_bir_lowering=False, debug=False,
                   num_devices=N_CORES)

    # ---- external parameters ----
    xt_d = nc.declare_dram_parameter("xt", [DIM, FLAT], bf16, isOutput=False)
    wqt_d = nc.declare_dram_parameter("wqt", [DIM, OC], bf16, isOutput=False)
    wkt_d = nc.declare_dram_parameter("wkt", [DIM, OC], bf16, isOutput=False)
    wvt_d = nc.declare_dram_parameter("wvt", [DIM, OC], bf16, isOutput=False)
    wot_d = nc.declare_dram_parameter("wot", [OC, DIM], bf16, isOutput=False)
    cos_d = nc.declare_dram_parameter("cos_t", [HD, S], bf16, isOutput=False)
    sin_d = nc.declare_dram_parameter("sin_t", [HD, S], bf16, isOutput=False)
    mdg_d = nc.declare_dram_parameter("mask_diag", [NQT, 128, 128], fp32, isOutput=False)
    rot_d = nc.declare_dram_parameter("rotp", [128, 128], bf16, isOutput=False)
    out_d = nc.declare_dram_parameter("out", [FLAT // N_CORES, DIM], bf16,
                                      isOutput=True)

    # ---- internal DRAM (o-proj partials + RS outputs) ----
    par_d = [nc.dram_tensor(f"partial_dram{c}", [RS_ROWS, DIM], bf16)
             for c in range(NCH)]
    rs_d = [nc.dram_tensor(f"rs_out{c}", [RS_ROWS // N_CORES, DIM], bf16)
            for c in range(NCH)]

    from contextlib import ExitStack
    with tile.TileContext(nc) as tc:
        with ExitStack() as _stk:
            cpool = _stk.enter_context(tc.tile_pool(name="consts", bufs=1))
            qkvres = _stk.enter_context(tc.tile_pool(name="qkvres", bufs=1))
            xpool = _stk.enter_context(tc.tile_pool(name="xT", bufs=33))
            spool = _stk.enter_context(tc.tile_pool(name="cops", bufs=4))
            ptpool = _stk.enter_context(tc.tile_pool(name="probsT", bufs=3))
            accpool = _stk.enter_context(tc.tile_pool(name="accs", bufs=1))
            smpool = _stk.enter_context(tc.tile_pool(name="small", bufs=4))
            opool = _stk.enter_context(tc.tile_pool(name="outT", bufs=2))
            papool = _stk.enter_context(tc.tile_pool(name="partial", bufs=2))
            qkvps = _stk.enter_context(
                tc.tile_pool(name="qkvps", bufs=2, space="PSUM"))
            workps = _stk.enter_context(
                tc.tile_pool(name="workps", bufs=4, space="PSUM"))
            pops = _stk.enter_context(
                tc.tile_pool(name="pops", bufs=1, space="PSUM"))

            # ---- consts (gpsimd DMA queue; cheap triggers) ----
            wot_sb = cpool.tile([128, HPC, DIM], bf16)
            nc.gpsimd.dma_start(wot_sb[:], wot_d[:].rearrange("(h p) e -> p h e", p=128))
            cos_sb = cpool.tile([HD, S], bf16)
            nc.gpsimd.dma_start(cos_sb[:], cos_d[:])
            sin_sb = cpool.tile([HD, S], bf16)
            nc.gpsimd.dma_start(sin_sb[:], sin_d[:])
            mdg_sb = cpool.tile([128, NQT, 128], fp32)
            nc.gpsimd.dma_start(mdg_sb[:], mdg_d[:].rearrange("t p k -> p t k"))
            rot_sb = cpool.tile([128, 128], bf16)
            nc.gpsimd.dma_start(rot_sb[:], rot_d[:])

            # ---- weights + first x chunk, interleaved in consumption order
            w_sb = {}
            for nm in ("q", "k", "v"):
                w_sb[nm] = qkvres.tile([128, DT, OC], bf16, tag=f"w{nm}", name=f"w{nm}")

            xts = {}  # (sc, dt) -> tile

            def load_x(sc):
                for dt in range(DT):
                    xt = xpool.tile([128, SC_W], bf16, tag="xt", name=f"xt{sc}_{dt}")
                    nc.sync.dma_start(
                        xt[:], xt_d[dt * 128:(dt + 1) * 128,
                                    sc * SC_W:(sc + 1) * SC_W])
                    xts[(sc, dt)] = xt

            for dt in range(DT):
                nc.sync.dma_start(w_sb["q"][:, dt, :],
                                  wqt_d[dt * 128:(dt + 1) * 128, :])
                xt = xpool.tile([128, SC_W], bf16, tag="xt", name=f"xt0_{dt}")
                nc.sync.dma_start(xt[:], xt_d[dt * 128:(dt + 1) * 128, 0:SC_W])
                xts[(0, dt)] = xt
            for dt in range(DT):
                nc.sync.dma_start(w_sb["k"][:, dt, :],
                                  wkt_d[dt * 128:(dt + 1) * 128, :])
            for dt in range(DT):
                nc.sync.dma_start(w_sb["v"][:, dt, :],
                                  wvt_d[dt * 128:(dt + 1) * 128, :])

            # ---- SBUF-resident q/k/v per (batch, head) ----
            qT = {(b, h): qkvres.tile([128, S], bf16, tag=f"qT{b}{h}", name=f"qT{b}{h}")
                  for b in range(B) for h in range(HPC)}
            kT = {(b, h): qkvres.tile([128, S], bf16, tag=f"kT{b}{h}", name=f"kT{b}{h}")
                  for b in range(B) for h in range(HPC)}
            vN = {(b, h): qkvres.tile([128, NQT, HD], bf16, tag=f"vN{b}{h}",
                                      name=f"vN{b}{h}")
                  for b in range(B) for h in range(HPC)}

            # =========== emission helpers ===========

            def emit_qkv_chunk(sc):
                """QKV projections + RoPE for one 512-col schunk.

                V chains first so the vb copies sit at the head of the
                scalar queue (their PSUM chains finish earliest) and the
                attention exps emitted right after this chunk aren't
                blocked behind a late PSUM drain.  The rope matmul of each
                q/k chain is emitted one chain later so its scalar-copy
                input is ready without stalling PE.
                """
                bb, c0 = divmod(sc * SC_W, S)
                s0 = c0  # position offset within batch
                chains = [(t, h) for t in ("v", "q", "k") for h in range(HPC)]
                pend = []  # rope matmuls pending emission: (t, h, til)
                for ci, (t, h) in enumerate(chains):
                    ps = qkvps.tile([128, SC_W], fp32, tag="qkv", name=f"ps_{t}{h}")
                    for dt in range(DT):
                        nc.tensor.matmul(
                            ps[:],
                            w_sb[t][:, dt, h * HD:(h + 1) * HD],
                            xts[(sc, dt)][:],
                            start=(dt == 0), stop=(dt == DT - 1))
                    if t in ("q", "k"):
                        # PSUM -> SBUF bf16 (+1/sqrt(hd) for q)
                        til = spool.tile([128, SC_W], bf16, tag="til", name=f"til{t}{h}")
                        nc.scalar.activation(til[:], ps[:], Copy,
                                             scale=SCALE if t == "q" else 1.0)
                        pend.append((t, h, til))
                    else:
                        vb = spool.tile([128, SC_W], bf16, tag="vb", name=f"vb{h}")
                        nc.scalar.copy(vb[:], ps[:])
                        for vt in range(SC_W // 128):
                            ktile = c0 // 128 + vt
                            nc.sync.dma_start_transpose(
                                vN[(bb, h)][:, ktile, :],
                                vb[:, vt * 128:(vt + 1) * 128])
                    if len(pend) > 1:
                        _emit_rope(bb, s0, *pend.pop(0))
                for args in pend:
                    _emit_rope(bb, s0, *args)

            def _emit_rope(bb, s0, t, h, til):
                rp = workps.tile([128, SC_W], fp32, tag="work", name=f"rot{t}{h}")
                nc.tensor.matmul(rp[:], rot_sb[:], til[:], start=True, stop=True)
                dst = qT[(bb, h)] if t == "q" else kT[(bb, h)]
                t1 = spool.tile([128, SC_W], bf16, tag="t1", name=f"t1{t}{h}")
                nc.vector.tensor_mul(t1[:], til[:], cos_sb[:, s0:s0 + SC_W])
                hat = spool.tile([128, SC_W], bf16, tag="hat", name=f"hat{t}{h}")
                nc.vector.tensor_mul(hat[:], rp[:], sin_sb[:, s0:s0 + SC_W])
                nc.vector.tensor_add(dst[:, s0:s0 + SC_W], hat[:], t1[:])

            oT_of = {}  # unit -> {h: oT tile}

            def emit_attn_unit(bb, qg):
                """Attention for 512 q-cols (group qg) of batch bb."""
                kmax = qg * 4 + 3
                po = {h: pops.tile([128, 512], fp32, tag=f"po{h}", name=f"po{h}")
                      for h in range(HPC)}
                acc_v = {h: accpool.tile([128, 512], bf16, tag=f"av{h}",
                                         name=f"accv{h}") for h in range(HPC)}
                acc_g = {h: accpool.tile([128, 512], bf16, tag=f"ag{h}",
                                         name=f"accg{h}") for h in range(HPC)}
                pt_hist = {h: {} for h in range(HPC)}

                def rowsum(h, kt, qlo, n):
                    # qg==0 has shrinking windows from kt=1 on; keep those
                    # units entirely on the vector accumulator.
                    pt = pt_hist[h][kt]
                    if qg == 0:
                        if kt == 0:
                            nc.vector.tensor_copy(acc_v[h][:], pt[:, :n])
                        else:
                            nc.vector.tensor_add(acc_v[h][:, qlo:512],
                                                 acc_v[h][:, qlo:512], pt[:, :n])
                        return
                    if kt == 0:
                        nc.vector.tensor_copy(acc_v[h][:], pt[:, :n])
                    elif kt == 1:
                        nc.gpsimd.tensor_copy(acc_g[h][:], pt[:, :n])
                    elif kt % 2 == 0:
                        nc.vector.tensor_add(acc_v[h][:, qlo:512],
                                             acc_v[h][:, qlo:512], pt[:, :n])
                    else:
                        nc.gpsimd.tensor_add(acc_g[h][:, qlo:512],
                                             acc_g[h][:, qlo:512], pt[:, :n])

                for kt in range(kmax + 1):
                    qlo = max(0, kt - qg * 4) * 128
                    n = 512 - qlo
                    for h in range(HPC):
                        sp = workps.tile([128, 512], fp32, tag="work", name="sp")
                        nc.tensor.matmul(
                            sp[:, :n],
                            kT[(bb, h)][:, kt * 128:(kt + 1) * 128],
                            qT[(bb, h)][:, qg * 512 + qlo:(qg + 1) * 512],
                            start=True, stop=True)
                        if kt >= qg * 4:  # diagonal block: causal mask
                            nc.vector.tensor_add(
                                sp[:, 0:128], sp[:, 0:128], mdg_sb[:, kt, :])
                        pt = ptpool.tile([128, 512], bf16, tag=f"pT{h}",
                                         name=f"pT{h}")
                        pt_hist[h][kt] = pt
                        nc.scalar.activation(pt[:, :n], sp[:, :n], Exp)
                        rowsum(h, kt, qlo, n)
                    if kt >= 1:
                        kl = kt - 1
                        ql2 = max(0, kl - qg * 4) * 128
                        n2 = 512 - ql2
                        for h in range(HPC):
                            nc.tensor.matmul(
                                po[h][:, ql2:512], vN[(bb, h)][:, kl, :],
                                pt_hist[h][kl][:, :n2],
                                start=(kl == 0), stop=False)
                for h in range(HPC):
                    nc.tensor.matmul(
                        po[h][:, 384:512], vN[(bb, h)][:, kmax, :],
                        pt_hist[h][kmax][:, :128], start=False, stop=True)

                # softmax denominators off the critical PE path
                oT_of[(bb, qg)] = {}
                for h in range(HPC):
                    if qg > 0:
                        nc.vector.tensor_add(acc_v[h][:], acc_v[h][:],
                                             acc_g[h][:])
                    sums = smpool.tile([128, 512], fp32, tag="sums",
                                       name="sums", bufs=2)
                    nc.gpsimd.partition_all_reduce(sums[:], acc_v[h][:],
                                                   channels=128,
                                                   reduce_op=RADD)
                    rbc = smpool.tile([128, 512], fp32, tag="rbc", name="rbc",
                                      bufs=2)
                    nc.vector.reciprocal_approx_fast(rbc[:], sums[:])
                    ot = opool.tile([128, 512], bf16, tag=f"oT{h}", name=f"oT{h}")
                    nc.vector.tensor_mul(ot[:], po[h][:], rbc[:])
                    oT_of[(bb, qg)][h] = ot

            def emit_oproj(bb, qg):
                """O-projection + ReduceScatter for one 512-row unit."""
                ot = oT_of.pop((bb, qg))
                chx = bb * NQG + qg
                for st in range(4):
                    par = papool.tile([128, DIM], bf16, tag="par", name="par")
                    for ec in range(4):
                        pp = workps.tile([128, 512], fp32, tag="work", name="pp")
                        for h in range(HPC):
                            nc.tensor.matmul(
                                pp[:],
                                ot[h][:, st * 128:(st + 1) * 128],
                                wot_sb[:, h, ec * 512:(ec + 1) * 512],
                                start=(h == 0), stop=(h == HPC - 1))
                        if ec % 2 == 0:
                            nc.scalar.copy(par[:, ec * 512:(ec + 1) * 512], pp[:])
                        else:
                            nc.vector.tensor_copy(par[:, ec * 512:(ec + 1) * 512], pp[:])
                    nc.sync.dma_start(par_d[chx][st * 128:(st + 1) * 128, :], par[:])
                nc.gpsimd.collective_compute(
                    "ReduceScatter", ADD, replica_groups=rg,
                    ins=[par_d[chx][:]],
                    outs=[rs_d[chx][:]])

            # =========== main schedule ===========
            # [QKV sc][attn unit(sc)][oproj unit(sc-1)]: o-proj lags one
            # unit so PE never waits on the softmax-normalize chain, and
            # its par copies queue on scalar/vector AFTER the attention
            # exps they'd otherwise delay.
            prev_unit = None
            for sc in range(NSC):
                if sc + 1 < NSC:
                    load_x(sc + 1)
                emit_qkv_chunk(sc)
                bb, qg = sc // NQG, sc % NQG
                emit_attn_unit(bb, qg)
                if prev_unit is not None:
                    emit_oproj(*prev_unit)
                prev_unit = (bb, qg)
            emit_oproj(*prev_unit)
            # rs -> out copies all at the end: chunks 0..6 are long done
            # (no queue blocking); only chunk 7's copy rides the RS tail.
            for chx in range(NCH):
                nc.gpsimd.dma_start(out_d[chx * 64:(chx + 1) * 64, :],
                                    rs_d[chx][:])

    nc.compile()
    return nc


def _get_nc(S):
    if S not in _CACHE:
        _CACHE[S] = _build(S)
    return _CACHE[S]


def make_inputs(x, freqs_cis, mask, wq, wk, wv, wo):
    """Host-side sharding / layout prep. Returns in_maps for 8 cores."""
    S = x.shape[1]
    flat_xt = np.ascontiguousarray(np.asarray(x, np.float32).reshape(B * S, DIM).T)
    cos = np.asarray(freqs_cis[..., 0], np.float32)   # [S, HD/2]
    sin = np.asarray(freqs_cis[..., 1], np.float32)
    cos_t = np.ascontiguousarray(np.repeat(cos.T, 2, axis=0))  # [HD, S]
    sin_t = np.ascontiguousarray(np.repeat(sin.T, 2, axis=0))
    m = np.asarray(mask, np.float32)[0, 0]
    nqt = S // 128
    mask_diag = np.ascontiguousarray(
        np.stack([m[i * 128:(i + 1) * 128, i * 128:(i + 1) * 128].T
                  for i in range(nqt)]))
    import ml_dtypes
    bf = ml_dtypes.bfloat16
    flat_xt = flat_xt.astype(bf)
    cos_t = cos_t.astype(bf)
    sin_t = sin_t.astype(bf)
    P = np.zeros((128, 128), np.float32)
    for j in range(64):
        P[2 * j, 2 * j + 1] = -1.0
        P[2 * j + 1, 2 * j] = 1.0
    rotp = np.ascontiguousarray(P.T)

    in_maps = []
    for c in range(N_CORES):
        r = slice(c * OC, (c + 1) * OC)
        in_maps.append({
            "xt": flat_xt,
            "wqt": np.ascontiguousarray(np.asarray(wq, np.float32)[r, :].T).astype(bf),
            "wkt": np.ascontiguousarray(np.asarray(wk, np.float32)[r, :].T).astype(bf),
            "wvt": np.ascontiguousarray(np.asarray(wv, np.float32)[r, :].T).astype(bf),
            "wot": np.ascontiguousarray(np.asarray(wo, np.float32)[:, r].T).astype(bf),
            "cos_t": cos_t,
            "sin_t": sin_t,
            "mask_diag": mask_diag,
            "rotp": rotp.astype(bf),
        })
    return in_maps


def assemble(results, S):
    """Undo the per-core ReduceScatter sharding into the full output."""
    nch = B * S // RS_ROWS
    full = np.empty((B * S, DIM), np.float32)
    for c in range(N_CORES):
        o = np.asarray(results[c]["out"], np.float32)  # [512, DIM]
        for chx in range(nch):
            full[chx * 512 + c * 64:chx * 512 + (c + 1) * 64] = \
                o[chx * 64:(chx + 1) * 64]
    return full.reshape(B, S, DIM)


def kernel(x, start_pos, freqs_cis, mask, wq, wk, wv, wo):
    from concourse.bass_utils import run_bass_kernel_spmd
    S = x.shape[1]
    nc = _get_nc(S)
    in_maps = make_inputs(x, freqs_cis, mask, wq, wk, wv, wo)
    res = run_bass_kernel_spmd(nc, in_maps, core_ids=list(range(N_CORES)))
    return assemble(res.results, S)


# revision 27
# speedup vs baseline: 1.1305x; 1.0088x over previous
"""Trainium2 8-core tensor-parallel attention kernel (Bass/Tile).

Sharding: heads tensor-parallel across 8 cores (2 heads/core).
wq/wk/wv column-sharded by head, wo row-sharded; x replicated.
Chunked ReduceScatter (bf16) after the output projection; the host
concatenates the per-core row shards into the full output.

Fused single-phase design: Q/K/V stay SBUF-resident (no DRAM
roundtrip), attention for q-group g of batch b is emitted right after
the schunk that completes its K/V prefix, o-proj lags one unit so PE
never waits on the softmax-normalize chain, softmax row-sums run on
vector+gpsimd (not PE), V is transposed with DMA-transpose, and the
final ReduceScatter chunk is split 4x to shrink the drain tail.

Self-contained: hardcodes B=2, S=2048, DIM=2048, NH=16, HD=128.
"""
import math

import numpy as np

B, S_FULL, DIM, NH = 2, 2048, 2048, 16
HD = 128
N_CORES = 8
HPC = NH // N_CORES          # heads per core
OC = HPC * HD                # output channels per core (256)
DT = DIM // 128              # d-tiles (16)
SC_W = 512                   # schunk width (cols of flattened seq)
RS_ROWS = 512                # rows per ReduceScatter chunk

_CACHE = {}


def _build(S):
    """Build the 8-core SPMD Bass graph for sequence length S (B=2 fixed)."""
    import concourse.bass as bass
    import concourse.mybir as mybir
    import concourse.tile as tile
    from concourse import bacc

    from concourse import bass_isa

    fp32 = mybir.dt.float32
    bf16 = mybir.dt.bfloat16
    Exp = mybir.ActivationFunctionType.Exp
    Copy = mybir.ActivationFunctionType.Copy
    ADD = mybir.AluOpType.add
    RADD = bass_isa.ReduceOp.add

    FLAT = B * S                 # flattened rows (4096)
    NSC = FLAT // SC_W           # schunks (8)
    NQT = S // 128               # q/k tiles per batch (16)
    NQG = NQT // 4               # 512-col q-groups per batch (4)
    NCH = FLAT // RS_ROWS        # ReduceScatter chunks (8)
    SCALE = 1.0 / math.sqrt(HD)
    rg = [list(range(N_CORES))]

    nc = bacc.Bacc("TRN2", target_bir_lowering=False, debug=False,
                   num_devices=N_CORES)

    # ---- external parameters ----
    xt_d = nc.declare_dram_parameter("xt", [DIM, FLAT], bf16, isOutput=False)
    wqt_d = nc.declare_dram_parameter("wqt", [DIM, OC], bf16, isOutput=False)
    wkt_d = nc.declare_dram_parameter("wkt", [DIM, OC], bf16, isOutput=False)
    wvt_d = nc.declare_dram_parameter("wvt", [DIM, OC], bf16, isOutput=False)
    wot_d = nc.declare_dram_parameter("wot", [OC, DIM], bf16, isOutput=False)
    cos_d = nc.declare_dram_parameter("cos_t", [HD, S], bf16, isOutput=False)
    sin_d = nc.declare_dram_parameter("sin_t", [HD, S], bf16, isOutput=False)
    mdg_d = nc.declare_dram_parameter("mask_diag", [NQT, 128, 128], fp32, isOutput=False)
    rot_d = nc.declare_dram_parameter("rotp", [128, 128], bf16, isOutput=False)
    out_d = nc.declare_dram_parameter("out", [FLAT // N_CORES, DIM], bf16,
                                      isOutput=True)

    # ---- internal DRAM (o-proj partials + RS outputs) ----
    par_d = [nc.dram_tensor(f"partial_dram{c}", [RS_ROWS, DIM], bf16)
             for c in range(NCH)]
    rs_d = [nc.dram_tensor(f"rs_out{c}", [RS_ROWS // N_CORES, DIM], bf16)
            for c in range(NCH)]

    from contextlib import ExitStack
    with tile.TileContext(nc) as tc:
        with ExitStack() as _stk:
            cpool = _stk.enter_context(tc.tile_pool(name="consts", bufs=1))
            qkvres = _stk.enter_context(tc.tile_pool(name="qkvres", bufs=1))
            xpool = _stk.enter_context(tc.tile_pool(name="xT", bufs=2))
            spool = _stk.enter_context(tc.tile_pool(name="cops", bufs=4))
            ptpool = _stk.enter_context(tc.tile_pool(name="probsT", bufs=3))
            accpool = _stk.enter_context(tc.tile_pool(name="accs", bufs=1))
            smpool = _stk.enter_context(tc.tile_pool(name="small", bufs=4))
            opool = _stk.enter_context(tc.tile_pool(name="outT", bufs=2))
            papool = _stk.enter_context(tc.tile_pool(name="partial", bufs=2))
            qkvps = _stk.enter_context(
                tc.tile_pool(name="qkvps", bufs=2, space="PSUM"))
            workps = _stk.enter_context(
                tc.tile_pool(name="workps", bufs=4, space="PSUM"))
            pops = _stk.enter_context(
                tc.tile_pool(name="pops", bufs=1, space="PSUM"))

            # ---- consts (gpsimd DMA queue; cheap triggers) ----
            wot_sb = cpool.tile([128, HPC, DIM], bf16)
            nc.gpsimd.dma_start(wot_sb[:], wot_d[:].rearrange("(h p) e -> p h e", p=128))
            cos_sb = cpool.tile([HD, S], bf16)
            nc.gpsimd.dma_start(cos_sb[:], cos_d[:])
            sin_sb = cpool.tile([HD, S], bf16)
            nc.gpsimd.dma_start(sin_sb[:], sin_d[:])
            mdg_sb = cpool.tile([128, NQT, 128], fp32)
            nc.gpsimd.dma_start(mdg_sb[:], mdg_d[:].rearrange("t p k -> p t k"))
            rot_sb = cpool.tile([128, 128], bf16)
            nc.gpsimd.dma_start(rot_sb[:], rot_d[:])

            # ---- weights + first x chunk, interleaved in consumption order
            w_sb = {}
            for nm in ("q", "k", "v"):
                w_sb[nm] = qkvres.tile([128, DT, OC], bf16, tag=f"w{nm}", name=f"w{nm}")

            xts = {}  # sc -> [128, DT, SC_W] tile

            def load_x(sc):
                # one strided DMA per chunk: 16 separate transfers would eat
                # ~9us of the sync sequencer per chunk (565ns/trigger)
                xt = xpool.tile([128, DT, SC_W], bf16, tag="xt", name=f"xt{sc}")
                nc.sync.dma_start(
                    xt[:], xt_d[:, sc * SC_W:(sc + 1) * SC_W]
                    .rearrange("(t p) c -> p t c", p=128))
                xts[sc] = xt

            # v chains run first: wv before wq before wk
            for dt in range(DT):
                nc.sync.dma_start(w_sb["v"][:, dt, :],
                                  wvt_d[dt * 128:(dt + 1) * 128, :])
                if dt == 0:
                    load_x(0)
            for dt in range(DT):
                nc.sync.dma_start(w_sb["q"][:, dt, :],
                                  wqt_d[dt * 128:(dt + 1) * 128, :])
            for dt in range(DT):
                nc.sync.dma_start(w_sb["k"][:, dt, :],
                                  wkt_d[dt * 128:(dt + 1) * 128, :])

            # ---- SBUF-resident q/k/v per (batch, head) ----
            qT = {(b, h): qkvres.tile([128, S], bf16, tag=f"qT{b}{h}", name=f"qT{b}{h}")
                  for b in range(B) for h in range(HPC)}
            kT = {(b, h): qkvres.tile([128, S], bf16, tag=f"kT{b}{h}", name=f"kT{b}{h}")
                  for b in range(B) for h in range(HPC)}
            vN = {(b, h): qkvres.tile([128, NQT, HD], bf16, tag=f"vN{b}{h}",
                                      name=f"vN{b}{h}")
                  for b in range(B) for h in range(HPC)}

            # =========== emission helpers ===========

            def emit_qkv_chunk(sc):
                """QKV projections + RoPE for one 512-col schunk.

                V chains first so the vb copies sit at the head of the
                scalar queue (their PSUM chains finish earliest) and the
                attention exps emitted right after this chunk aren't
                blocked behind a late PSUM drain.  The rope matmul of each
                q/k chain is emitted one chain later so its scalar-copy
                input is ready without stalling PE.
                """
                bb, c0 = divmod(sc * SC_W, S)
                s0 = c0  # position offset within batch
                chains = [(t, h) for t in ("v", "q", "k") for h in range(HPC)]
                pend = []  # rope matmuls pending emission: (t, h, til)
                for ci, (t, h) in enumerate(chains):
                    ps = qkvps.tile([128, SC_W], fp32, tag="qkv", name=f"ps_{t}{h}")
                    for dt in range(DT):
                        nc.tensor.matmul(
                            ps[:],
                            w_sb[t][:, dt, h * HD:(h + 1) * HD],
                            xts[sc][:, dt, :],
                            start=(dt == 0), stop=(dt == DT - 1))
                    if t in ("q", "k"):
                        # PSUM -> SBUF bf16 (+1/sqrt(hd) for q)
                        til = spool.tile([128, SC_W], bf16, tag="til", name=f"til{t}{h}")
                        nc.scalar.activation(til[:], ps[:], Copy,
                                             scale=SCALE if t == "q" else 1.0)
                        pend.append((t, h, til))
                    else:
                        vb = spool.tile([128, SC_W], bf16, tag="vb", name=f"vb{h}")
                        nc.scalar.copy(vb[:], ps[:])
                        kt0 = c0 // 128
                        nc.sync.dma_start_transpose(
                            vN[(bb, h)][:, kt0:kt0 + 4, :], vb[:])
                    if len(pend) > 1:
                        _emit_rope(bb, s0, *pend.pop(0))
                for args in pend:
                    _emit_rope(bb, s0, *args)

            def _emit_rope(bb, s0, t, h, til):
                rp = workps.tile([128, SC_W], fp32, tag="work", name=f"rot{t}{h}")
                nc.tensor.matmul(rp[:], rot_sb[:], til[:], start=True, stop=True)
                dst = qT[(bb, h)] if t == "q" else kT[(bb, h)]
                t1 = spool.tile([128, SC_W], bf16, tag="t1", name=f"t1{t}{h}")
                nc.vector.tensor_mul(t1[:], til[:], cos_sb[:, s0:s0 + SC_W])
                hat = spool.tile([128, SC_W], bf16, tag="hat", name=f"hat{t}{h}")
                nc.vector.tensor_mul(hat[:], rp[:], sin_sb[:, s0:s0 + SC_W])
                nc.vector.tensor_add(dst[:, s0:s0 + SC_W], hat[:], t1[:])

            oT_of = {}  # unit -> {h: oT tile}

            def emit_attn_unit(bb, qg):
                """Attention for 512 q-cols (group qg) of batch bb."""
                kmax = qg * 4 + 3
                po = {h: pops.tile([128, 512], fp32, tag=f"po{h}", name=f"po{h}")
                      for h in range(HPC)}
                acc_v = {h: accpool.tile([128, 512], bf16, tag=f"av{h}",
                                         name=f"accv{h}") for h in range(HPC)}
                acc_g = {h: accpool.tile([128, 512], bf16, tag=f"ag{h}",
                                         name=f"accg{h}") for h in range(HPC)}
                pt_hist = {h: {} for h in range(HPC)}

                def rowsum(h, kt, qlo, n):
                    # qg==0 has shrinking windows from kt=1 on; keep those
                    # units entirely on the vector accumulator.
                    pt = pt_hist[h][kt]
                    if qg == 0:
                        if kt == 0:
                            nc.vector.tensor_copy(acc_v[h][:], pt[:, :n])
                        else:
                            nc.vector.tensor_add(acc_v[h][:, qlo:512],
                                                 acc_v[h][:, qlo:512], pt[:, :n])
                        return
                    if kt == 0:
                        nc.vector.tensor_copy(acc_v[h][:], pt[:, :n])
                    elif kt == 1:
                        nc.gpsimd.tensor_copy(acc_g[h][:], pt[:, :n])
                    elif kt % 2 == 0:
                        nc.vector.tensor_add(acc_v[h][:, qlo:512],
                                             acc_v[h][:, qlo:512], pt[:, :n])
                    else:
                        nc.gpsimd.tensor_add(acc_g[h][:, qlo:512],
                                             acc_g[h][:, qlo:512], pt[:, :n])

                for kt in range(kmax + 1):
                    qlo = max(0, kt - qg * 4) * 128
                    n = 512 - qlo
                    for h in range(HPC):
                        sp = workps.tile([128, 512], fp32, tag="work", name="sp")
                        nc.tensor.matmul(
                            sp[:, :n],
                            kT[(bb, h)][:, kt * 128:(kt + 1) * 128],
                            qT[(bb, h)][:, qg * 512 + qlo:(qg + 1) * 512],
                            start=True, stop=True)
                        if kt >= qg * 4:  # diagonal block: causal mask
                            nc.vector.tensor_add(
                                sp[:, 0:128], sp[:, 0:128], mdg_sb[:, kt, :])
                        pt = ptpool.tile([128, 512], bf16, tag=f"pT{h}",
                                         name=f"pT{h}")
                        pt_hist[h][kt] = pt
                        nc.scalar.activation(pt[:, :n], sp[:, :n], Exp)
                        rowsum(h, kt, qlo, n)
                    if kt >= 1:
                        kl = kt - 1
                        ql2 = max(0, kl - qg * 4) * 128
                        n2 = 512 - ql2
                        for h in range(HPC):
                            nc.tensor.matmul(
                                po[h][:, ql2:512], vN[(bb, h)][:, kl, :],
                                pt_hist[h][kl][:, :n2],
                                start=(kl == 0), stop=False)
                for h in range(HPC):
                    nc.tensor.matmul(
                        po[h][:, 384:512], vN[(bb, h)][:, kmax, :],
                        pt_hist[h][kmax][:, :128], start=False, stop=True)

                # softmax denominators off the critical PE path
                oT_of[(bb, qg)] = {}
                for h in range(HPC):
                    if qg > 0:
                        nc.vector.tensor_add(acc_v[h][:], acc_v[h][:],
                                             acc_g[h][:])
                    sums = smpool.tile([128, 512], fp32, tag="sums",
                                       name="sums", bufs=2)
                    nc.gpsimd.partition_all_reduce(sums[:], acc_v[h][:],
                                                   channels=128,
                                                   reduce_op=RADD)
                    rbc = smpool.tile([128, 512], fp32, tag="rbc", name="rbc",
                                      bufs=2)
                    nc.vector.reciprocal_approx_fast(rbc[:], sums[:])
                    ot = opool.tile([128, 512], bf16, tag=f"oT{h}", name=f"oT{h}")
                    nc.vector.tensor_mul(ot[:], po[h][:], rbc[:])
                    oT_of[(bb, qg)][h] = ot

            def emit_oproj(bb, qg):
                """O-projection + ReduceScatter for one 512-row unit."""
                ot = oT_of.pop((bb, qg))
                chx = bb * NQG + qg
                for st in range(4):
                    par = papool.tile([128, DIM], bf16, tag="par", name="par")
                    for ec in range(4):
                        pp = workps.tile([128, 512], fp32, tag="work", name="pp")
                        for h in range(HPC):
                            nc.tensor.matmul(
                                pp[:],
                                ot[h][:, st * 128:(st + 1) * 128],
                                wot_sb[:, h, ec * 512:(ec + 1) * 512],
                                start=(h == 0), stop=(h == HPC - 1))
                        if ec % 2 == 0:
                            nc.scalar.copy(par[:, ec * 512:(ec + 1) * 512], pp[:])
                        else:
                            nc.vector.tensor_copy(par[:, ec * 512:(ec + 1) * 512], pp[:])
                    nc.sync.dma_start(par_d[chx][st * 128:(st + 1) * 128, :], par[:])
                nc.gpsimd.collective_compute(
                    "ReduceScatter", ADD, replica_groups=rg,
                    ins=[par_d[chx][:]],
                    outs=[rs_d[chx][:]])

            # =========== main schedule ===========
            # [QKV sc][attn unit(sc)][oproj unit(sc-1)]: o-proj lags one
            # unit so PE never waits on the softmax-normalize chain, and
            # its par copies queue on scalar/vector AFTER the attention
            # exps they'd otherwise delay.
            prev_unit = None
            for sc in range(NSC):
                if sc + 1 < NSC:
                    load_x(sc + 1)
                emit_qkv_chunk(sc)
                bb, qg = sc // NQG, sc % NQG
                emit_attn_unit(bb, qg)
                if prev_unit is not None:
                    emit_oproj(*prev_unit)
                prev_unit = (bb, qg)
            emit_oproj(*prev_unit)
            # rs -> out copies all at the end: chunks 0..6 are long done
            # (no queue blocking); only chunk 7's copy rides the RS tail.
            for chx in range(NCH):
                nc.gpsimd.dma_start(out_d[chx * 64:(chx + 1) * 64, :],
                                    rs_d[chx][:])

    nc.compile()
    return nc


def _get_nc(S):
    if S not in _CACHE:
        _CACHE[S] = _build(S)
    return _CACHE[S]


def make_inputs(x, freqs_cis, mask, wq, wk, wv, wo):
    """Host-side sharding / layout prep. Returns in_maps for 8 cores."""
    S = x.shape[1]
    flat_xt = np.ascontiguousarray(np.asarray(x, np.float32).reshape(B * S, DIM).T)
    cos = np.asarray(freqs_cis[..., 0], np.float32)   # [S, HD/2]
    sin = np.asarray(freqs_cis[..., 1], np.float32)
    cos_t = np.ascontiguousarray(np.repeat(cos.T, 2, axis=0))  # [HD, S]
    sin_t = np.ascontiguousarray(np.repeat(sin.T, 2, axis=0))
    m = np.asarray(mask, np.float32)[0, 0]
    nqt = S // 128
    mask_diag = np.ascontiguousarray(
        np.stack([m[i * 128:(i + 1) * 128, i * 128:(i + 1) * 128].T
                  for i in range(nqt)]))
    import ml_dtypes
    bf = ml_dtypes.bfloat16
    flat_xt = flat_xt.astype(bf)
    cos_t = cos_t.astype(bf)
    sin_t = sin_t.astype(bf)
    P = np.zeros((128, 128), np.float32)
    for j in range(64):
        P[2 * j, 2 * j + 1] = -1.0
        P[2 * j + 1, 2 * j] = 1.0
    rotp = np.ascontiguousarray(P.T)

    in_maps = []
    for c in range(N_CORES):
        r = slice(c * OC, (c + 1) * OC)
        in_maps.append({
            "xt": flat_xt,
            "wqt": np.ascontiguousarray(np.asarray(wq, np.float32)[r, :].T).astype(bf),
            "wkt": np.ascontiguousarray(np.asarray(wk, np.float32)[r, :].T).astype(bf),
            "wvt": np.ascontiguousarray(np.asarray(wv, np.float32)[r, :].T).astype(bf),
            "wot": np.ascontiguousarray(np.asarray(wo, np.float32)[:, r].T).astype(bf),
            "cos_t": cos_t,
            "sin_t": sin_t,
            "mask_diag": mask_diag,
            "rotp": rotp.astype(bf),
        })
    return in_maps


def assemble(results, S):
    """Undo the per-core ReduceScatter sharding into the full output."""
    nch = B * S // RS_ROWS
    full = np.empty((B * S, DIM), np.float32)
    for c in range(N_CORES):
        o = np.asarray(results[c]["out"], np.float32)  # [512, DIM]
        for chx in range(nch):
            full[chx * 512 + c * 64:chx * 512 + (c + 1) * 64] = \
                o[chx * 64:(chx + 1) * 64]
    return full.reshape(B, S, DIM)


def kernel(x, start_pos, freqs_cis, mask, wq, wk, wv, wo):
    from concourse.bass_utils import run_bass_kernel_spmd
    S = x.shape[1]
    nc = _get_nc(S)
    in_maps = make_inputs(x, freqs_cis, mask, wq, wk, wv, wo)
    res = run_bass_kernel_spmd(nc, in_maps, core_ids=list(range(N_CORES)))
    return assemble(res.results, S)


# revision 29
# speedup vs baseline: 1.3519x; 1.1958x over previous
"""Trainium2 8-core tensor-parallel attention kernel (Bass/Tile).

Sharding: heads tensor-parallel across 8 cores (2 heads/core).
wq/wk/wv column-sharded by head, wo row-sharded; x replicated.
Chunked ReduceScatter (bf16) after the output projection; the host
concatenates the per-core row shards into the full output.

Fused single-phase design with software pipelining:
  iteration sc emits, finely interleaved on the PE stream,
    - QKV projections + RoPE for schunk sc,
    - attention for the unit completed by schunk sc-1,
    - o-projection (+ ReduceScatter) for the unit before that,
  so the Tensor engine always has independent matmuls in flight while
  the Activation engine works through the softmax exps.  Q/K/V stay
  SBUF-resident, softmax row-sums run on vector (bf16) with a gpsimd
  partition_all_reduce, V is transposed with one DMA-transpose per
  chain, and all bulk loads are single partition-major DMAs.

Self-contained: hardcodes B=2, S=2048, DIM=2048, NH=16, HD=128.
"""
import math

import numpy as np

B, S_FULL, DIM, NH = 2, 2048, 2048, 16
HD = 128
N_CORES = 8
HPC = NH // N_CORES          # heads per core
OC = HPC * HD                # output channels per core (256)
DT = DIM // 128              # d-tiles (16)
SC_W = 512                   # schunk width (cols of flattened seq)
RS_ROWS = 512                # rows per ReduceScatter chunk

_CACHE = {}


def _build(S):
    """Build the 8-core SPMD Bass graph for sequence length S (B=2 fixed)."""
    import concourse.bass as bass
    import concourse.mybir as mybir
    import concourse.tile as tile
    from concourse import bacc
    from concourse import bass_isa

    fp32 = mybir.dt.float32
    bf16 = mybir.dt.bfloat16
    Exp = mybir.ActivationFunctionType.Exp
    Copy = mybir.ActivationFunctionType.Copy
    ADD = mybir.AluOpType.add
    RADD = bass_isa.ReduceOp.add

    FLAT = B * S                 # flattened rows (4096)
    NSC = FLAT // SC_W           # schunks (8)
    NQT = S // 128               # q/k tiles per batch (16)
    NQG = NQT // 4               # 512-col q-groups per batch (4)
    NCH = FLAT // RS_ROWS        # ReduceScatter chunks (8)
    SCALE = 1.0 / math.sqrt(HD)
    rg = [list(range(N_CORES))]

    nc = bacc.Bacc("TRN2", target_bir_lowering=False, debug=False,
                   num_devices=N_CORES)

    # ---- external parameters (partition-major for big-descriptor DMAs) ----
    xch_d = nc.declare_dram_parameter("xch", [NSC, 128, DT * SC_W], bf16,
                                      isOutput=False)
    wqp_d = nc.declare_dram_parameter("wqp", [128, DT * OC], bf16, isOutput=False)
    wkp_d = nc.declare_dram_parameter("wkp", [128, DT * OC], bf16, isOutput=False)
    wvp_d = nc.declare_dram_parameter("wvp", [128, DT * OC], bf16, isOutput=False)
    wop_d = nc.declare_dram_parameter("wop", [128, HPC * DIM], bf16, isOutput=False)
    cos_d = nc.declare_dram_parameter("cos_t", [HD, S], bf16, isOutput=False)
    sin_d = nc.declare_dram_parameter("sin_t", [HD, S], bf16, isOutput=False)
    mdg_d = nc.declare_dram_parameter("mdgp", [128, NQT * 128], fp32, isOutput=False)
    rot_d = nc.declare_dram_parameter("rotp", [128, 128], bf16, isOutput=False)
    out_d = nc.declare_dram_parameter("out", [FLAT // N_CORES, DIM], bf16,
                                      isOutput=True)

    # ---- internal DRAM (o-proj partials + RS outputs) ----
    par_d = [nc.dram_tensor(f"partial_dram{c}", [RS_ROWS, DIM], bf16)
             for c in range(NCH)]
    rs_d = [nc.dram_tensor(f"rs_out{c}", [RS_ROWS // N_CORES, DIM], bf16)
            for c in range(NCH)]

    from contextlib import ExitStack
    with tile.TileContext(nc) as tc:
        with ExitStack() as _stk:
            cpool = _stk.enter_context(tc.tile_pool(name="consts", bufs=1))
            qkvres = _stk.enter_context(tc.tile_pool(name="qkvres", bufs=1))
            xpool = _stk.enter_context(tc.tile_pool(name="xT", bufs=2))
            spool = _stk.enter_context(tc.tile_pool(name="cops", bufs=4))
            ptpool = _stk.enter_context(tc.tile_pool(name="probsT", bufs=3))
            accpool = _stk.enter_context(tc.tile_pool(name="accs", bufs=1))
            smpool = _stk.enter_context(tc.tile_pool(name="small", bufs=2))
            opool = _stk.enter_context(tc.tile_pool(name="outT", bufs=2))
            papool = _stk.enter_context(tc.tile_pool(name="partial", bufs=2))
            qkvps = _stk.enter_context(
                tc.tile_pool(name="qkvps", bufs=2, space="PSUM"))
            scps = _stk.enter_context(
                tc.tile_pool(name="scps", bufs=2, space="PSUM"))
            wkps = _stk.enter_context(
                tc.tile_pool(name="wkps", bufs=2, space="PSUM"))
            pops = _stk.enter_context(
                tc.tile_pool(name="pops", bufs=1, space="PSUM"))

            # ---- consts (gpsimd queue) ----
            wot_sb = cpool.tile([128, HPC, DIM], bf16)
            nc.gpsimd.dma_start(
                wot_sb[:], wop_d[:].rearrange("p (h e) -> p h e", h=HPC))
            cos_sb = cpool.tile([HD, S], bf16)
            nc.gpsimd.dma_start(cos_sb[:], cos_d[:])
            sin_sb = cpool.tile([HD, S], bf16)
            nc.gpsimd.dma_start(sin_sb[:], sin_d[:])
            mdg_sb = cpool.tile([128, NQT, 128], fp32)
            nc.gpsimd.dma_start(
                mdg_sb[:], mdg_d[:].rearrange("p (t k) -> p t k", t=NQT))
            rot_sb = cpool.tile([128, 128], bf16)
            nc.gpsimd.dma_start(rot_sb[:], rot_d[:])

            # ---- weights + x chunks: one partition-major DMA each ----
            w_sb = {}
            xts = {}

            def load_x(sc):
                xt = xpool.tile([128, DT, SC_W], bf16, tag="xt", name=f"xt{sc}")
                nc.sync.dma_start(
                    xt[:], xch_d[sc].rearrange("p (t c) -> p t c", t=DT))
                xts[sc] = xt

            for nm, src in (("v", wvp_d), ("q", wqp_d), ("k", wkp_d)):
                w_sb[nm] = qkvres.tile([128, DT, OC], bf16, tag=f"w{nm}",
                                       name=f"w{nm}")
                nc.sync.dma_start(
                    w_sb[nm][:], src[:].rearrange("p (t e) -> p t e", t=DT))
                if nm == "v":
                    load_x(0)

            # ---- SBUF-resident q/k/v per (batch, head) ----
            qT = {(b, h): qkvres.tile([128, S], bf16, tag=f"qT{b}{h}", name=f"qT{b}{h}")
                  for b in range(B) for h in range(HPC)}
            kT = {(b, h): qkvres.tile([128, S], bf16, tag=f"kT{b}{h}", name=f"kT{b}{h}")
                  for b in range(B) for h in range(HPC)}
            vN = {(b, h): qkvres.tile([128, NQT, HD], bf16, tag=f"vN{b}{h}",
                                      name=f"vN{b}{h}")
                  for b in range(B) for h in range(HPC)}

            oT_of = {}

            # =========== emission generators ===========

            def _emit_rope(bb, s0, t, h, til):
                rp = wkps.tile([128, SC_W], fp32, tag="wk", name=f"rot{t}{h}")
                nc.tensor.matmul(rp[:], rot_sb[:], til[:], start=True, stop=True)
                dst = qT[(bb, h)] if t == "q" else kT[(bb, h)]
                t1 = spool.tile([128, SC_W], bf16, tag="t1", name=f"t1{t}{h}")
                nc.vector.tensor_mul(t1[:], til[:], cos_sb[:, s0:s0 + SC_W])
                hat = spool.tile([128, SC_W], bf16, tag="hat", name=f"hat{t}{h}")
                nc.vector.tensor_mul(hat[:], rp[:], sin_sb[:, s0:s0 + SC_W])
                nc.vector.tensor_add(dst[:, s0:s0 + SC_W], hat[:], t1[:])

            def chunk_gen(sc):
                """QKV projections + RoPE for one schunk; yields per chain."""
                bb, c0 = divmod(sc * SC_W, S)
                s0 = c0
                chains = [(t, h) for t in ("v", "q", "k") for h in range(HPC)]
                pend = []
                for ci, (t, h) in enumerate(chains):
                    ps = qkvps.tile([128, SC_W], fp32, tag="qkv", name=f"ps_{t}{h}")
                    for dt in range(DT):
                        nc.tensor.matmul(
                            ps[:],
                            w_sb[t][:, dt, h * HD:(h + 1) * HD],
                            xts[sc][:, dt, :],
                            start=(dt == 0), stop=(dt == DT - 1))
                    if t in ("q", "k"):
                        til = spool.tile([128, SC_W], bf16, tag="til",
                                         name=f"til{t}{h}")
                        nc.scalar.activation(til[:], ps[:], Copy,
                                             scale=SCALE if t == "q" else 1.0)
                        pend.append((t, h, til))
                    else:
                        vb = spool.tile([128, SC_W], bf16, tag="vb", name=f"vb{h}")
                        nc.scalar.copy(vb[:], ps[:])
                        kt0 = c0 // 128
                        nc.sync.dma_start_transpose(
                            vN[(bb, h)][:, kt0:kt0 + 4, :], vb[:])
                    if len(pend) > 1:
                        _emit_rope(bb, s0, *pend.pop(0))
                    yield
                for args in pend:
                    _emit_rope(bb, s0, *args)

            def attn_gen(bb, qg):
                """Attention for 512 q-cols (group qg); yields per kt step."""
                kmax = qg * 4 + 3
                po = {h: pops.tile([128, 512], fp32, tag=f"po{h}", name=f"po{h}")
                      for h in range(HPC)}
                acc_v = {h: accpool.tile([128, 512], bf16, tag=f"av{h}",
                                         name=f"accv{h}") for h in range(HPC)}
                acc_g = {h: accpool.tile([128, 512], bf16, tag=f"ag{h}",
                                         name=f"accg{h}") for h in range(HPC)}
                pt_hist = {h: {} for h in range(HPC)}

                def rowsum(h, kt, qlo, n):
                    pt = pt_hist[h][kt]
                    if qg == 0:
                        if kt == 0:
                            nc.vector.tensor_copy(acc_v[h][:], pt[:, :n])
                        else:
                            nc.vector.tensor_add(acc_v[h][:, qlo:512],
                                                 acc_v[h][:, qlo:512], pt[:, :n])
                        return
                    if kt == 0:
                        nc.vector.tensor_copy(acc_v[h][:], pt[:, :n])
                    elif kt == 1:
                        nc.vector.tensor_copy(acc_g[h][:], pt[:, :n])
                    elif kt % 2 == 0:
                        nc.vector.tensor_add(acc_v[h][:, qlo:512],
                                             acc_v[h][:, qlo:512], pt[:, :n])
                    else:
                        nc.vector.tensor_add(acc_g[h][:, qlo:512],
                                             acc_g[h][:, qlo:512], pt[:, :n])

                for kt in range(kmax + 1):
                    qlo = max(0, kt - qg * 4) * 128
                    n = 512 - qlo
                    for h in range(HPC):
                        sp = scps.tile([128, 512], fp32, tag="sc", name="sp")
                        nc.tensor.matmul(
                            sp[:, :n],
                            kT[(bb, h)][:, kt * 128:(kt + 1) * 128],
                            qT[(bb, h)][:, qg * 512 + qlo:(qg + 1) * 512],
                            start=True, stop=True)
                        if kt >= qg * 4:  # diagonal block: causal mask
                            nc.vector.tensor_add(
                                sp[:, 0:128], sp[:, 0:128], mdg_sb[:, kt, :])
                        pt = ptpool.tile([128, 512], bf16, tag=f"pT{h}",
                                         name=f"pT{h}")
                        pt_hist[h][kt] = pt
                        nc.scalar.activation(pt[:, :n], sp[:, :n], Exp)
                        rowsum(h, kt, qlo, n)
                    if kt >= 1:
                        kl = kt - 1
                        ql2 = max(0, kl - qg * 4) * 128
                        n2 = 512 - ql2
                        for h in range(HPC):
                            nc.tensor.matmul(
                                po[h][:, ql2:512], vN[(bb, h)][:, kl, :],
                                pt_hist[h][kl][:, :n2],
                                start=(kl == 0), stop=False)
                    yield
                for h in range(HPC):
                    nc.tensor.matmul(
                        po[h][:, 384:512], vN[(bb, h)][:, kmax, :],
                        pt_hist[h][kmax][:, :128], start=False, stop=True)

                # softmax denominators off the critical PE path
                oT_of[(bb, qg)] = {}
                for h in range(HPC):
                    if qg > 0:
                        nc.vector.tensor_add(acc_v[h][:], acc_v[h][:],
                                             acc_g[h][:])
                    sums = smpool.tile([128, 512], fp32, tag="sums", name="sums")
                    nc.gpsimd.partition_all_reduce(sums[:], acc_v[h][:],
                                                   channels=128,
                                                   reduce_op=RADD)
                    rbc = smpool.tile([128, 512], fp32, tag="rbc", name="rbc")
                    nc.vector.reciprocal_approx_fast(rbc[:], sums[:])
                    ot = opool.tile([128, 512], bf16, tag=f"oT{h}", name=f"oT{h}")
                    nc.vector.tensor_mul(ot[:], po[h][:], rbc[:])
                    oT_of[(bb, qg)][h] = ot

            def oproj_gen(bb, qg):
                """O-projection + ReduceScatter for one unit; yields per st."""
                ot = oT_of.pop((bb, qg))
                chx = bb * NQG + qg
                for st in range(4):
                    par = papool.tile([128, DIM], bf16, tag="par", name="par")
                    for ec in range(4):
                        pp = wkps.tile([128, 512], fp32, tag="wk", name="pp")
                        for h in range(HPC):
                            nc.tensor.matmul(
                                pp[:],
                                ot[h][:, st * 128:(st + 1) * 128],
                                wot_sb[:, h, ec * 512:(ec + 1) * 512],
                                start=(h == 0), stop=(h == HPC - 1))
                        if ec % 2 == 0:
                            nc.scalar.copy(par[:, ec * 512:(ec + 1) * 512], pp[:])
                        else:
                            nc.vector.tensor_copy(
                                par[:, ec * 512:(ec + 1) * 512], pp[:])
                    nc.sync.dma_start(par_d[chx][st * 128:(st + 1) * 128, :],
                                      par[:])
                    if st < 3:
                        yield
                nc.gpsimd.collective_compute(
                    "ReduceScatter", ADD, replica_groups=rg,
                    ins=[par_d[chx][:]], outs=[rs_d[chx][:]])

            def pump(gen, n):
                """Advance gen up to n steps; return True when exhausted."""
                if gen is None:
                    return True
                for _ in range(n):
                    if next(gen, _DONE) is _DONE:
                        return True
                return False

            _DONE = object()

            def drain(ag, og):
                a_done = ag is None
                o_done = og is None
                while not (a_done and o_done):
                    if not a_done:
                        a_done = pump(ag, 2)
                    if not o_done:
                        o_done = pump(og, 1)

            # =========== main schedule ===========
            # iteration sc: chunk(sc) ⊗ attn(unit sc-1) ⊗ oproj(unit sc-2)
            units = [(s // NQG, s % NQG) for s in range(NSC)]
            for sc in range(NSC):
                if sc + 1 < NSC:
                    load_x(sc + 1)
                cg = chunk_gen(sc)
                ag = attn_gen(*units[sc - 1]) if sc >= 1 else None
                og = oproj_gen(*units[sc - 2]) if sc >= 2 else None
                a_steps = (units[sc - 1][1] * 4 + 4) if ag else 0
                done_a = ag is None
                done_o = og is None
                for ci in range(6):
                    pump(cg, 1)
                    if not done_o and ci >= 1:
                        done_o = pump(og, 1)
                    if not done_a:
                        share = max(1, -(-a_steps // 6))
                        done_a = pump(ag, share)
                pump(cg, 1)  # run the trailing rope flush
                drain(None if done_a else ag, None if done_o else og)
            # post-loop: attn of last unit ⊗ oproj of second-to-last
            ag = attn_gen(*units[NSC - 1])
            og = oproj_gen(*units[NSC - 2])
            drain(ag, og)
            og = oproj_gen(*units[NSC - 1])
            drain(None, og)
            # rs -> out copies: chunks 0..6 long done (no queue blocking);
            # chunk 7's copy rides the RS tail.
            for chx in range(NCH):
                nc.gpsimd.dma_start(out_d[chx * 64:(chx + 1) * 64, :],
                                    rs_d[chx][:])

    nc.compile()
    return nc


def _get_nc(S):
    if S not in _CACHE:
        _CACHE[S] = _build(S)
    return _CACHE[S]


def make_inputs(x, freqs_cis, mask, wq, wk, wv, wo):
    """Host-side sharding / layout prep. Returns in_maps for 8 cores."""
    import ml_dtypes
    bf = ml_dtypes.bfloat16
    S = x.shape[1]
    FLAT = B * S
    NSC = FLAT // SC_W

    flat_xt = np.asarray(x, np.float32).reshape(FLAT, DIM).T  # [DIM, FLAT]
    # chunk-partition-major: xch[sc, p, t*SC_W + c] = xt[t*128+p, sc*SC_W+c]
    xch = np.ascontiguousarray(
        flat_xt.reshape(DT, 128, FLAT)[:, :, :]
        .transpose(1, 0, 2)            # [128, DT, FLAT]
        .reshape(128, DT, NSC, SC_W)
        .transpose(2, 0, 1, 3)         # [NSC, 128, DT, SC_W]
        .reshape(NSC, 128, DT * SC_W)).astype(bf)

    cos = np.asarray(freqs_cis[..., 0], np.float32)   # [S, HD/2]
    sin = np.asarray(freqs_cis[..., 1], np.float32)
    cos_t = np.ascontiguousarray(np.repeat(cos.T, 2, axis=0)).astype(bf)
    sin_t = np.ascontiguousarray(np.repeat(sin.T, 2, axis=0)).astype(bf)
    m = np.asarray(mask, np.float32)[0, 0]
    nqt = S // 128
    mask_diag = np.stack([m[i * 128:(i + 1) * 128, i * 128:(i + 1) * 128].T
                          for i in range(nqt)])       # [NQT, 128, 128]
    mdgp = np.ascontiguousarray(
        mask_diag.transpose(1, 0, 2).reshape(128, nqt * 128))

    P = np.zeros((128, 128), np.float32)
    for j in range(64):
        P[2 * j, 2 * j + 1] = -1.0
        P[2 * j + 1, 2 * j] = 1.0
    rotp = np.ascontiguousarray(P.T).astype(bf)

    def pmajor(w):  # [DIM, OC] -> [128, DT*OC]
        return np.ascontiguousarray(
            w.reshape(DT, 128, -1).transpose(1, 0, 2).reshape(128, -1))

    in_maps = []
    for c in range(N_CORES):
        r = slice(c * OC, (c + 1) * OC)
        wqt = np.asarray(wq, np.float32)[r, :].T      # [DIM, OC]
        wkt = np.asarray(wk, np.float32)[r, :].T
        wvt = np.asarray(wv, np.float32)[r, :].T
        wot = np.asarray(wo, np.float32)[:, r].T      # [OC, DIM]
        wop = np.ascontiguousarray(
            wot.reshape(HPC, 128, DIM).transpose(1, 0, 2).reshape(128, -1))
        in_maps.append({
            "xch": xch,
            "wqp": pmajor(wqt).astype(bf),
            "wkp": pmajor(wkt).astype(bf),
            "wvp": pmajor(wvt).astype(bf),
            "wop": wop.astype(bf),
            "cos_t": cos_t,
            "sin_t": sin_t,
            "mdgp": mdgp,
            "rotp": rotp,
        })
    return in_maps


def assemble(results, S):
    """Undo the per-core ReduceScatter sharding into the full output."""
    nch = B * S // RS_ROWS
    full = np.empty((B * S, DIM), np.float32)
    for c in range(N_CORES):
        o = np.asarray(results[c]["out"], np.float32)  # [512, DIM]
        for chx in range(nch):
            full[chx * 512 + c * 64:chx * 512 + (c + 1) * 64] = \
                o[chx * 64:(chx + 1) * 64]
    return full.reshape(B, S, DIM)


def kernel(x, start_pos, freqs_cis, mask, wq, wk, wv, wo):
    from concourse.bass_utils import run_bass_kernel_spmd
    S = x.shape[1]
    nc = _get_nc(S)
    in_maps = make_inputs(x, freqs_cis, mask, wq, wk, wv, wo)
    res = run_bass_kernel_spmd(nc, in_maps, core_ids=list(range(N_CORES)))
    return assemble(res.results, S)
